# revision 10
# baseline (speedup 1.0000x reference)
"""Two-layer GCN (MultiOrderGraphLayer) Bass kernel for 8 Trainium2 cores.

Math: out = 0.5*(relu(A_hat@x@W1+b1) + relu(A_hat@x@W2+b2)) with
A_hat = D^-1/2 (A+I) D^-1/2.  Both layers share A_hat, so g = A_hat @ x is
computed once and the two small 128x128 matmuls are applied afterwards.

Normalization factorization: norm_e = dinv[src]*dinv[dst].  The host
pre-scales x rows by dinv (x' = dinv[i]*x[i], bf16), so gathered rows
already carry the src factor; the dst factor dinv[n] is applied in the
output stage as a per-partition activation scale (partition = node there).
Self-loops reduce to g_raw[:, n] += x'[n, :], added per window with one
identity matmul instead of gather slots.

Device algorithm (per core, feature-major g_T = [128 feat, nodes]):
  - nodes sharded 8 ways by row; edges partitioned by destination core and
    grouped per 128-node output window into variable-length cells packed
    back-to-back (cell length = max edge count across cores, so the SPMD
    program shape is shared; per-core shortfall is masked via dstloc=-1).
  - sources split into lo (<32768) / hi streams for int16 dma_gather.
  - the stream is consumed in fixed 128-slot blocks; for each (window,
    overlapping block) pair the host emits a dstloc column; slots outside
    the window (straddle or pad) carry -1 and the one-hot masks them out.
  - per block-use: build S[e, n] = (dstloc_e == n) in one DVE is_equal
    (bf16, GF columns fused); accumulate psum += xg^T @ S.
  - output: psum = g_T^T@(0.5W) + rdinv*(0.5b); out = relu(psum * dinv[n])
    (per-partition scale), layers averaged, written node-major.
"""

import math
import numpy as np

N_NODES = 50000
D = 128
N_CORES = 8
SPLIT = 32768  # int16 gather index limit
WIN = 128      # output-window size in nodes (one-hot width / psum free dim)
CHUNK = 4096   # slots per dma_gather instruction (multiple of 128)
N_QUEUES = 4   # SWDGE queues (ucode max); rotation parallelizes drain
GF = 8         # one-hot columns fused per DVE is_equal op
WARM = (1024, 1024, 2048, 2048)  # warmup chunks; keep in sync with idx split


# ---------------------------------------------------------------- host prep

def host_prep(edge_index, n_nodes, n_cores, split=SPLIT, chunk=CHUNK):
    """Edge partitioning by destination core, per-window cells (variable
    length, shared shape across cores), lo/hi source split, block/window
    overlap map, dstloc mask columns.

    All heavy math stays on device; host work is indexing plus the x
    prescale (done in make_core_inputs).
    """
    src = np.asarray(edge_index[0], dtype=np.int64)
    dst = np.asarray(edge_index[1], dtype=np.int64)
    deg = np.bincount(dst, minlength=n_nodes).astype(np.int64) + 1
    dinv = (1.0 / np.sqrt(deg.astype(np.float64))).astype(np.float32)

    npc = n_nodes // n_cores
    assert npc * n_cores == n_nodes
    nwin = math.ceil(npc / WIN)
    n_halves = 2 if n_nodes > split else 1

    per_core = []
    counts = np.zeros((n_cores, n_halves, nwin), np.int64)
    for c in range(n_cores):
        n0 = c * npc
        m = (dst >= n0) & (dst < n0 + npc)
        s, d = src[m], dst[m]
        w = (d - n0) // WIN
        half = (s >= split).astype(np.int64) if n_halves == 2 else np.zeros_like(s)
        key = half * nwin + w
        order = np.argsort(key, kind="stable")
        s, d, key = s[order], d[order], key[order]
        cnt = np.bincount(key, minlength=n_halves * nwin)
        counts[c] = cnt.reshape(n_halves, nwin)
        per_core.append((s, d, cnt))

    # shared cell lengths (max across cores) and packed offsets per half
    clen = counts.max(axis=0)                       # [n_halves, nwin]
    coff = np.zeros_like(clen)
    half_len = np.zeros(n_halves, np.int64)
    for h in range(n_halves):
        coff[h] = np.concatenate([[0], np.cumsum(clen[h])[:-1]])
        half_len[h] = -(-int(clen[h].sum()) // 128) * 128  # pad tail to 128

    # block/window overlap map (compile-time, shared across cores):
    # mm list per window = blocks intersecting [coff, coff+clen)
    mm_cols = []   # global column order: for w: lo block uses, hi block uses
    win_mms = [[] for _ in range(nwin)]
    for w in range(nwin):
        for h in range(n_halves):
            a, b = int(coff[h, w]), int(coff[h, w] + clen[h, w])
            if b == a:
                continue
            for blk in range(a // 128, -(-b // 128)):
                win_mms[w].append((h, blk, len(mm_cols)))
                mm_cols.append((h, blk, w))
    nmm = len(mm_cols)

    # chunk split per half: warmup small, bulk CHUNK, cooldown small
    chunk_sizes = []
    for h in range(n_halves):
        rem, sizes = int(half_len[h]), []
        for wsz in WARM:
            L = min(wsz, rem)
            if L > 0:
                sizes.append(L)
                rem -= L
        while rem > 0:
            L = min(chunk, rem)
            sizes.append(L)
            rem -= L
        # cooldown: resplit the trailing ~3-4K slots into small chunks so
        # the last windows' data lands early and the tail chain is short
        tail = 0
        while sizes and tail + sizes[-1] <= 4096 and sizes[-1] >= 1024:
            tail += sizes.pop()
        cool = []
        for csz in (1024, 1024, 512, 512, 512, 512):
            if tail >= csz + 128:
                cool.append(csz)
                tail -= csz
        if tail > 0:
            cool.append(tail)
        sizes += cool
        chunk_sizes.append(sizes)

    per_core_inputs = []
    for c in range(n_cores):
        s, d, cnt = per_core[c]
        offs = np.concatenate([[0], np.cumsum(cnt)])
        # slot streams per half: sources (idx) and per-slot local dst
        idx_stream = [np.zeros(int(half_len[h]), np.int64) for h in range(n_halves)]
        dst_stream = [np.full(int(half_len[h]), -1, np.int64) for h in range(n_halves)]
        for h in range(n_halves):
            for w in range(nwin):
                k = h * nwin + w
                a, b = int(offs[k]), int(offs[k + 1])
                o = int(coff[h, w])
                idx_stream[h][o:o + (b - a)] = s[a:b] - h * split
                dst_stream[h][o:o + (b - a)] = d[a:b] - c * npc - w * WIN

        # dstloc mask columns: one [128] column per (window, block) use;
        # slots outside the window's cell get -1
        dl = np.full((128, nmm), -1.0, np.float32)
        for col, (h, blk, w) in enumerate(mm_cols):
            s0, s1 = blk * 128, blk * 128 + 128
            a, b = int(coff[h, w]), int(coff[h, w] + clen[h, w])
            lo, hi = max(s0, a), min(s1, b)
            if hi > lo:
                seg = dst_stream[h][lo:hi].astype(np.float32)
                # mask slots whose dst is outside this window (pad slots
                # carry -1 already; straddle slots belong to w by range)
                dl[lo - s0:hi - s0, col] = seg
        core_in = {"dstloc": np.ascontiguousarray(dl)}

        # own-slab metadata for self-loops + output normalization
        nd = np.arange(npc, dtype=np.int64) + c * npc
        dv = dinv[nd]                                   # [npc]
        pad = nwin * WIN - npc
        dvp = np.concatenate([dv, np.zeros(pad, np.float32)])
        core_in["dinv_own"] = np.ascontiguousarray(
            dvp.reshape(nwin, WIN).T)                   # [128, nwin]
        core_in["rdinv"] = (1.0 / dvp.reshape(1, -1)[:, :npc].clip(1e-30)
                            ).astype(np.float32)        # [1, npc]

        # gather indices: wrapped [16, L/16] per chunk, replicated 8x
        for h in range(n_halves):
            cols, off = [], 0
            for L in chunk_sizes[h]:
                a = idx_stream[h][off:off + L].reshape(-1, 16).T
                cols.append(a)
                off += L
            wrapped = np.concatenate(cols, axis=1).astype(np.int16)
            core_in["idx_h%d" % h] = np.ascontiguousarray(
                np.tile(wrapped, (8, 1)))
        per_core_inputs.append(core_in)

    meta = dict(n_nodes=n_nodes, n_cores=n_cores, npc=npc, nwin=nwin,
                n_halves=n_halves, split=split, half_len=half_len,
                nmm=nmm, win_mms=win_mms, chunk=chunk,
                chunk_sizes=chunk_sizes, dinv=dinv)
    return meta, per_core_inputs


# ------------------------------------------------------------- bass program

def build_program(meta):
    import concourse.bacc as bacc
    import concourse.mybir as mybir
    import concourse.tile as tile
    from concourse import library_config

    f32 = mybir.dt.float32
    bf16 = mybir.dt.bfloat16
    i16 = mybir.dt.int16
    AF = mybir.ActivationFunctionType
    OP = mybir.AluOpType

    n_nodes = meta["n_nodes"]
    npc, nwin = meta["npc"], meta["nwin"]
    n_halves, split = meta["n_halves"], meta["split"]
    half_len = meta["half_len"]
    nmm = meta["nmm"]
    win_mms = meta["win_mms"]
    chunk = meta["chunk"]
    chunk_sizes = meta["chunk_sizes"]
    warm_tot = sum(WARM)

    nc = bacc.Bacc("TRN2", num_swdge_queues=N_QUEUES)

    x_d = nc.declare_dram_parameter("x", [n_nodes, D], bf16, isOutput=False)
    xo_d = nc.declare_dram_parameter("x_own", [npc, D], bf16, isOutput=False)
    dl_d = nc.declare_dram_parameter("dstloc", [128, nmm], bf16, isOutput=False)
    dvo_d = nc.declare_dram_parameter("dinv_own", [128, nwin], f32,
                                      isOutput=False)
    rd_d = nc.declare_dram_parameter("rdinv", [1, npc], bf16, isOutput=False)
    idx_d = [nc.declare_dram_parameter("idx_h%d" % h,
                                       [128, int(half_len[h]) // 16],
                                       i16, isOutput=False)
             for h in range(n_halves)]
    w1_d = nc.declare_dram_parameter("W1", [D, D], bf16, isOutput=False)
    w2_d = nc.declare_dram_parameter("W2", [D, D], bf16, isOutput=False)
    b1_d = nc.declare_dram_parameter("b1", [1, D], bf16, isOutput=False)
    b2_d = nc.declare_dram_parameter("b2", [1, D], bf16, isOutput=False)
    iota_d = nc.declare_dram_parameter("iota", [128, GF * 128], bf16,
                                       isOutput=False)
    id_d = nc.declare_dram_parameter("ident", [128, 128], bf16, isOutput=False)
    out_d = nc.declare_dram_parameter("out", [npc, D], f32, isOutput=True)

    WG = 4  # windows per phase-2 batch (one 512-wide psum bank)

    with tile.TileContext(nc) as tc:
        with (
            tc.tile_pool(name="const", bufs=1) as constp,
            tc.tile_pool(name="xg", bufs=8) as xgp,
            tc.tile_pool(name="eq", bufs=4) as eqp,
            tc.tile_pool(name="ps1", bufs=3, space="PSUM") as ps1,
            tc.tile_pool(name="ps2", bufs=2, space="PSUM") as ps2,
            tc.tile_pool(name="fin", bufs=3) as finp,
        ):
            # Q7 library holding DMAGatherAnt; must precede all gathers
            nc.gpsimd.load_library(library_config.mlp)

            # idx warmup slices first: the first gather depends only on
            # these, so they must not queue behind the big constant loads
            idx_all = []
            idx_rest = []
            for h in range(n_halves):
                cols = int(half_len[h]) // 16
                wcols = min(sum(WARM) // 16, cols)
                t = constp.tile([128, cols], i16, tag="idx%d" % h)
                nc.sync.dma_start(t[:, :wcols], idx_d[h][:, :wcols])
                if wcols < cols:
                    idx_rest.append((t, wcols, cols, h))
                idx_all.append(t)

            # --- constants / metadata (weights and biases arrive already
            # scaled by 0.5; x rows arrive scaled by dinv[src])
            iota8 = constp.tile([128, GF, 128], bf16)
            nc.sync.dma_start(
                iota8[:], iota_d[:].rearrange("p (c n) -> p c n", n=128))
            ident = constp.tile([128, 128], bf16)
            nc.sync.dma_start(ident[:], id_d[:])
            wts = {}
            for nm, src_d in (("w1", w1_d), ("w2", w2_d)):
                t = constp.tile([128, 128], bf16, tag=nm)
                nc.sync.dma_start(t[:], src_d[:])
                wts[nm] = t
            bias = {}
            for nm, src_d in (("b1", b1_d), ("b2", b2_d)):
                t = constp.tile([1, 128], bf16, tag=nm)
                nc.sync.dma_start(t[:], src_d[:])
                bias[nm] = t
            rdinv = constp.tile([1, npc], bf16)
            nc.sync.dma_start(rdinv[:], rd_d[:])
            dvo = constp.tile([128, nwin], f32)
            nc.sync.dma_start(dvo[:], dvo_d[:])

            dl = constp.tile([128, nmm], bf16)
            nc.sync.dma_start(dl[:], dl_d[:])

            for t, wcols, cols, h in idx_rest:
                nc.sync.dma_start(t[:, wcols:], idx_d[h][:, wcols:])

            # own slab, node-major per window: [128 node, nwin, 128 feat]
            xown = constp.tile([128, nwin, 128], bf16)
            nc.sync.dma_start(
                xown[:, :npc // 128, :],
                xo_d[: (npc // 128) * 128, :].rearrange(
                    "(w p) f -> p w f", p=128))
            if npc % 128:
                nc.sync.dma_start(
                    xown[: npc % 128, npc // 128, :],
                    xo_d[(npc // 128) * 128:, :])

            g_all = constp.tile([128, npc], bf16)

            # one-hot columns: GF fused per DVE is_equal; consumed strictly
            # in column order so a single active group suffices
            eq_cache = [None, None]  # [group id, tile]

            def get_eq(col):
                g = col // GF
                if eq_cache[0] != g:
                    g0 = g * GF
                    gl = min(GF, nmm - g0)
                    eq = eqp.tile([128, GF, 128], bf16, tag="eq")
                    nc.vector.tensor_tensor(
                        out=eq[:, :gl, :], in0=iota8[:, :gl, :],
                        in1=dl[:, g0:g0 + gl, None].to_broadcast([128, gl, 128]),
                        op=OP.is_equal)
                    eq_cache[0] = g
                    eq_cache[1] = eq
                return eq_cache[1]

            # per-half stream state: lazy chunk issuing in window order
            class Stream:
                pass

            streams = []
            for h in range(n_halves):
                s = Stream()
                s.h = h
                s.base = x_d[0:split, :] if h == 0 else x_d[split:n_nodes, :]
                s.chunk_bounds = []
                off = 0
                for L in chunk_sizes[h]:
                    s.chunk_bounds.append((off, L))
                    off += L
                s.blk2chunk = np.repeat(
                    np.arange(len(chunk_sizes[h])),
                    [L // 128 for L in chunk_sizes[h]])
                s.tiles = {}
                streams.append(s)

            ci_global = 0

            def ensure_chunk(s, ci):
                nonlocal ci_global
                if ci in s.tiles:
                    return s.tiles[ci]
                off, L = s.chunk_bounds[ci]
                xg = xgp.tile([128, chunk // 128, 128], bf16, tag="xg")
                nc.gpsimd.dma_gather(
                    out_ap=xg[:, : L // 128, :],
                    in_ap=s.base,
                    idxs_ap=idx_all[s.h][:, off // 16:(off + L) // 16],
                    num_idxs=L,
                    num_idxs_reg=L,
                    elem_size=D,
                    single_packet=False,
                    queue_num=ci_global % N_QUEUES,
                )
                ci_global += 1
                s.tiles.clear()
                s.tiles[ci] = xg
                return xg

            # --- output stage: psum = g^T@(W/2) + rdinv*(b/2), then
            # relu with per-partition dst scale dinv[n]; layers averaged
            def emit_phase2(wlo, whi):
                nwg = whi - wlo + 1
                wls = [min(WIN, npc - w * WIN) for w in range(wlo, whi + 1)]
                pps = {}
                for nm_w, nm_b in (("w1", "b1"), ("w2", "b2")):
                    pp = ps2.tile([128, WG * 128], f32, tag="pp")
                    for j, w in enumerate(range(wlo, whi + 1)):
                        wl = wls[j]
                        sl = pp[:wl, j * 128:(j + 1) * 128]
                        nc.tensor.matmul(sl, g_all[:, w * WIN:w * WIN + wl],
                                         wts[nm_w][:], start=True, stop=False)
                        nc.tensor.matmul(sl,
                                         rdinv[:, w * WIN:w * WIN + wl],
                                         bias[nm_b][:], start=False, stop=True)
                    o = finp.tile([128, WG, 128], f32, tag="o" + nm_w)
                    for j, w in enumerate(range(wlo, whi + 1)):
                        nc.scalar.activation(
                            o[:wls[j], j, :],
                            pp[:wls[j], j * 128:(j + 1) * 128], AF.Relu,
                            scale=dvo[:wls[j], w:w + 1])
                    pps[nm_w] = o
                ot = finp.tile([128, WG, 128], f32, tag="ot")
                rows = min(wls)
                otf = ot[:].rearrange("p c n -> p (c n)")
                o1f = pps["w1"][:].rearrange("p c n -> p (c n)")
                o2f = pps["w2"][:].rearrange("p c n -> p (c n)")
                if rows == 128:
                    nc.vector.tensor_tensor(otf[:, :nwg * 128],
                                            o1f[:, :nwg * 128],
                                            o2f[:, :nwg * 128], op=OP.add)
                else:
                    for j in range(nwg):
                        cs = slice(j * 128, j * 128 + 128)
                        nc.vector.tensor_tensor(otf[:wls[j], cs],
                                                o1f[:wls[j], cs],
                                                o2f[:wls[j], cs], op=OP.add)
                for j, w in enumerate(range(wlo, whi + 1)):
                    nc.sync.dma_start(out_d[w * WIN:w * WIN + wls[j], :],
                                      ot[:wls[j], j, :])

            for w in range(nwin):
                wlen = min(WIN, npc - w * WIN)
                pw = ps1.tile([128, 128], f32, tag="pw")
                mms = win_mms[w]
                n_tot = len(mms) + 1
                # self-loop first: x'_own rows -> columns via identity
                nc.tensor.matmul(pw[:, :wlen], xown[:wlen, w, :],
                                 ident[:wlen, :wlen],
                                 start=True, stop=(n_tot == 1))
                for k, (h, blk, col) in enumerate(mms):
                    s = streams[h]
                    ci = int(s.blk2chunk[blk])
                    xg = ensure_chunk(s, ci)
                    bl = blk - s.chunk_bounds[ci][0] // 128
                    eq = get_eq(col)
                    nc.tensor.matmul(
                        pw[:, :wlen],
                        xg[:, bl, :],
                        eq[:, col % GF, :wlen],
                        start=False,
                        stop=(k == n_tot - 2),
                    )
                nc.scalar.activation(g_all[:, w * WIN:w * WIN + wlen],
                                     pw[:, :wlen], AF.Copy)
                if w % WG == WG - 1 or w == nwin - 1:
                    emit_phase2(w - (w % WG), w)

    nc.compile()
    return nc


def make_core_inputs(meta, per_core_inputs, x, W1, b1, W2, b2):
    """Full in_maps for run_bass_kernel_spmd (adds shared tensors).

    x rows are pre-scaled by dinv[src] so gathered rows carry the source
    normalization; weights/biases fold in the 0.5 layer average.
    """
    import ml_dtypes
    bf = ml_dtypes.bfloat16
    dinv = meta["dinv"]
    npc = meta["npc"]
    xs = (np.asarray(x, np.float32) * dinv[:, None]).astype(bf)
    xs = np.ascontiguousarray(xs)
    shared = {
        "x": xs,
        "W1": np.ascontiguousarray((0.5 * np.asarray(W1, np.float32)).astype(bf)),
        "W2": np.ascontiguousarray((0.5 * np.asarray(W2, np.float32)).astype(bf)),
        "b1": (0.5 * np.asarray(b1, np.float32)).astype(bf).reshape(1, D),
        "b2": (0.5 * np.asarray(b2, np.float32)).astype(bf).reshape(1, D),
        "iota": np.ascontiguousarray(np.broadcast_to(
            np.tile(np.arange(128, dtype=np.float32), GF),
            (128, GF * 128)).astype(bf)),
        "ident": np.ascontiguousarray(np.eye(128, dtype=np.float32).astype(bf)),
    }
    maps = []
    for c, ci in enumerate(per_core_inputs):
        m = dict(shared)
        m["x_own"] = np.ascontiguousarray(xs[c * npc:(c + 1) * npc, :])
        m["dstloc"] = np.ascontiguousarray(ci["dstloc"].astype(bf))
        m["dinv_own"] = np.ascontiguousarray(ci["dinv_own"].astype(np.float32))
        m["rdinv"] = np.ascontiguousarray(ci["rdinv"].astype(bf))
        for k, v in ci.items():
            if k.startswith("idx_"):
                m[k] = v
        maps.append(m)
    return maps


# ------------------------------------------------------------------- kernel

def kernel(x, edge_index, W1, b1, W2, b2, _trace=False):
    from concourse.bass_utils import run_bass_kernel_spmd

    x = np.asarray(x)
    n_nodes = x.shape[0]
    meta, pci = host_prep(edge_index, n_nodes, N_CORES)
    nc = build_program(meta)
    in_maps = make_core_inputs(meta, pci, x, W1, b1, W2, b2)
    res = run_bass_kernel_spmd(nc, in_maps, list(range(N_CORES)),
                               trace=_trace)
    out = np.concatenate([res.results[c]["out"] for c in range(N_CORES)],
                         axis=0)
    if _trace:
        return out, res
    return out


# revision 13
# speedup vs baseline: 1.0094x; 1.0094x over previous
"""Two-layer GCN (MultiOrderGraphLayer) Bass kernel for 8 Trainium2 cores.

Math: out = 0.5*(relu(A_hat@x@W1+b1) + relu(A_hat@x@W2+b2)) with
A_hat = D^-1/2 (A+I) D^-1/2.  Both layers share A_hat, so g = A_hat @ x is
computed once and the two small 128x128 matmuls are applied afterwards.

Normalization factorization: norm_e = dinv[src]*dinv[dst].  The host
pre-scales x rows by dinv (x' = dinv[i]*x[i], bf16), so gathered rows
already carry the src factor; the dst factor dinv[n] is applied in the
output stage as a per-partition activation scale (partition = node there).
Self-loops reduce to g_raw[:, n] += x'[n, :], added per window with one
identity matmul instead of gather slots.

Device algorithm (per core, feature-major g_T = [128 feat, nodes]):
  - nodes sharded 8 ways by row; edges partitioned by destination core and
    grouped per 128-node output window into variable-length cells packed
    back-to-back (cell length = max edge count across cores, so the SPMD
    program shape is shared; per-core shortfall is masked via dstloc=-1).
  - sources split into lo (<32768) / hi streams for int16 dma_gather.
  - the stream is consumed in fixed 128-slot blocks; for each (window,
    overlapping block) pair the host emits a dstloc column; slots outside
    the window (straddle or pad) carry -1 and the one-hot masks them out.
  - per block-use: build S[e, n] = (dstloc_e == n) in one DVE is_equal
    (bf16, GF columns fused); accumulate psum += xg^T @ S.
  - output: psum = g_T^T@(0.5W) + rdinv*(0.5b); out = relu(psum * dinv[n])
    (per-partition scale), layers averaged, written node-major.
"""

import math
import numpy as np

N_NODES = 50000
D = 128
N_CORES = 8
SPLIT = 32768  # int16 gather index limit
WIN = 128      # output-window size in nodes (one-hot width / psum free dim)
CHUNK = 4096   # slots per dma_gather instruction (multiple of 128)
N_QUEUES = 4   # SWDGE queues (ucode max); rotation parallelizes drain
GF = 8         # one-hot columns fused per DVE is_equal op
WARM = (1024, 1024, 2048, 2048)  # warmup chunks; keep in sync with idx split


# ---------------------------------------------------------------- host prep

def host_prep(edge_index, n_nodes, n_cores, split=SPLIT, chunk=CHUNK):
    """Edge partitioning by destination core, per-window cells (variable
    length, shared shape across cores), lo/hi source split, block/window
    overlap map, dstloc mask columns.

    All heavy math stays on device; host work is indexing plus the x
    prescale (done in make_core_inputs).
    """
    src = np.asarray(edge_index[0], dtype=np.int64)
    dst = np.asarray(edge_index[1], dtype=np.int64)
    deg = np.bincount(dst, minlength=n_nodes).astype(np.int64) + 1
    dinv = (1.0 / np.sqrt(deg.astype(np.float64))).astype(np.float32)

    npc = n_nodes // n_cores
    assert npc * n_cores == n_nodes
    nwin = math.ceil(npc / WIN)
    n_halves = 2 if n_nodes > split else 1

    per_core = []
    counts = np.zeros((n_cores, n_halves, nwin), np.int64)
    for c in range(n_cores):
        n0 = c * npc
        m = (dst >= n0) & (dst < n0 + npc)
        s, d = src[m], dst[m]
        w = (d - n0) // WIN
        half = (s >= split).astype(np.int64) if n_halves == 2 else np.zeros_like(s)
        key = half * nwin + w
        order = np.argsort(key, kind="stable")
        s, d, key = s[order], d[order], key[order]
        cnt = np.bincount(key, minlength=n_halves * nwin)
        counts[c] = cnt.reshape(n_halves, nwin)
        per_core.append((s, d, cnt))

    # shared cell lengths (max across cores) and packed offsets per half
    clen = counts.max(axis=0)                       # [n_halves, nwin]
    coff = np.zeros_like(clen)
    half_len = np.zeros(n_halves, np.int64)
    for h in range(n_halves):
        coff[h] = np.concatenate([[0], np.cumsum(clen[h])[:-1]])
        half_len[h] = -(-int(clen[h].sum()) // 128) * 128  # pad tail to 128

    # block/window overlap map (compile-time, shared across cores):
    # mm list per window = blocks intersecting [coff, coff+clen)
    mm_cols = []   # global column order: for w: lo block uses, hi block uses
    win_mms = [[] for _ in range(nwin)]
    for w in range(nwin):
        for h in range(n_halves):
            a, b = int(coff[h, w]), int(coff[h, w] + clen[h, w])
            if b == a:
                continue
            for blk in range(a // 128, -(-b // 128)):
                win_mms[w].append((h, blk, len(mm_cols)))
                mm_cols.append((h, blk, w))
    nmm = len(mm_cols)

    # chunk split per half: warmup small, bulk CHUNK, cooldown small
    chunk_sizes = []
    for h in range(n_halves):
        rem, sizes = int(half_len[h]), []
        for wsz in WARM:
            L = min(wsz, rem)
            if L > 0:
                sizes.append(L)
                rem -= L
        while rem > 0:
            L = min(chunk, rem)
            sizes.append(L)
            rem -= L
        # cooldown: resplit the trailing ~3-4K slots into small chunks so
        # the last windows' data lands early and the tail chain is short
        tail = 0
        while sizes and tail + sizes[-1] <= 4096 and sizes[-1] >= 1024:
            tail += sizes.pop()
        cool = []
        for csz in (1024, 1024, 512, 512, 512, 512):
            if tail >= csz + 128:
                cool.append(csz)
                tail -= csz
        if tail > 0:
            cool.append(tail)
        sizes += cool
        chunk_sizes.append(sizes)

    per_core_inputs = []
    for c in range(n_cores):
        s, d, cnt = per_core[c]
        offs = np.concatenate([[0], np.cumsum(cnt)])
        # slot streams per half: sources (idx) and per-slot local dst
        idx_stream = [np.zeros(int(half_len[h]), np.int64) for h in range(n_halves)]
        dst_stream = [np.full(int(half_len[h]), -1, np.int64) for h in range(n_halves)]
        for h in range(n_halves):
            for w in range(nwin):
                k = h * nwin + w
                a, b = int(offs[k]), int(offs[k + 1])
                o = int(coff[h, w])
                idx_stream[h][o:o + (b - a)] = s[a:b] - h * split
                dst_stream[h][o:o + (b - a)] = d[a:b] - c * npc - w * WIN

        # dstloc mask columns: one [128] column per (window, block) use;
        # slots outside the window's cell get -1
        dl = np.full((128, nmm), -1.0, np.float32)
        for col, (h, blk, w) in enumerate(mm_cols):
            s0, s1 = blk * 128, blk * 128 + 128
            a, b = int(coff[h, w]), int(coff[h, w] + clen[h, w])
            lo, hi = max(s0, a), min(s1, b)
            if hi > lo:
                seg = dst_stream[h][lo:hi].astype(np.float32)
                # mask slots whose dst is outside this window (pad slots
                # carry -1 already; straddle slots belong to w by range)
                dl[lo - s0:hi - s0, col] = seg
        core_in = {"dstloc": np.ascontiguousarray(dl)}

        # own-slab metadata for self-loops + output normalization
        nd = np.arange(npc, dtype=np.int64) + c * npc
        dv = dinv[nd]                                   # [npc]
        pad = nwin * WIN - npc
        dvp = np.concatenate([dv, np.zeros(pad, np.float32)])
        core_in["dinv_own"] = np.ascontiguousarray(
            dvp.reshape(nwin, WIN).T)                   # [128, nwin]
        core_in["rdinv"] = (1.0 / dvp.reshape(1, -1)[:, :npc].clip(1e-30)
                            ).astype(np.float32)        # [1, npc]

        # gather indices: wrapped [16, L/16] per chunk, replicated 8x
        for h in range(n_halves):
            cols, off = [], 0
            for L in chunk_sizes[h]:
                a = idx_stream[h][off:off + L].reshape(-1, 16).T
                cols.append(a)
                off += L
            wrapped = np.concatenate(cols, axis=1).astype(np.int16)
            core_in["idx_h%d" % h] = np.ascontiguousarray(
                np.tile(wrapped, (8, 1)))
        per_core_inputs.append(core_in)

    meta = dict(n_nodes=n_nodes, n_cores=n_cores, npc=npc, nwin=nwin,
                n_halves=n_halves, split=split, half_len=half_len,
                nmm=nmm, win_mms=win_mms, chunk=chunk,
                chunk_sizes=chunk_sizes, dinv=dinv)
    return meta, per_core_inputs


# ------------------------------------------------------------- bass program

def build_program(meta):
    import concourse.bacc as bacc
    import concourse.mybir as mybir
    import concourse.tile as tile
    from concourse import library_config

    f32 = mybir.dt.float32
    bf16 = mybir.dt.bfloat16
    i16 = mybir.dt.int16
    AF = mybir.ActivationFunctionType
    OP = mybir.AluOpType

    n_nodes = meta["n_nodes"]
    npc, nwin = meta["npc"], meta["nwin"]
    n_halves, split = meta["n_halves"], meta["split"]
    half_len = meta["half_len"]
    nmm = meta["nmm"]
    win_mms = meta["win_mms"]
    chunk = meta["chunk"]
    chunk_sizes = meta["chunk_sizes"]
    warm_tot = sum(WARM)

    nc = bacc.Bacc("TRN2", num_swdge_queues=N_QUEUES)

    x_d = nc.declare_dram_parameter("x", [n_nodes, D], bf16, isOutput=False)
    xo_d = nc.declare_dram_parameter("x_own", [npc, D], bf16, isOutput=False)
    dl_d = nc.declare_dram_parameter("dstloc", [128, nmm], bf16, isOutput=False)
    dvo_d = nc.declare_dram_parameter("dinv_own", [128, nwin], f32,
                                      isOutput=False)
    rd_d = nc.declare_dram_parameter("rdinv", [1, npc], bf16, isOutput=False)
    idx_d = [nc.declare_dram_parameter("idx_h%d" % h,
                                       [128, int(half_len[h]) // 16],
                                       i16, isOutput=False)
             for h in range(n_halves)]
    w1_d = nc.declare_dram_parameter("W1", [D, D], bf16, isOutput=False)
    w2_d = nc.declare_dram_parameter("W2", [D, D], bf16, isOutput=False)
    b1_d = nc.declare_dram_parameter("b1", [1, D], bf16, isOutput=False)
    b2_d = nc.declare_dram_parameter("b2", [1, D], bf16, isOutput=False)
    iota_d = nc.declare_dram_parameter("iota", [128, GF * 128], bf16,
                                       isOutput=False)
    id_d = nc.declare_dram_parameter("ident", [128, 128], bf16, isOutput=False)
    out_d = nc.declare_dram_parameter("out", [npc, D], f32, isOutput=True)

    WG = 4  # windows per phase-2 batch (one 512-wide psum bank)

    with tile.TileContext(nc) as tc:
        with (
            tc.tile_pool(name="const", bufs=1) as constp,
            tc.tile_pool(name="xg", bufs=8) as xgp,
            tc.tile_pool(name="eq", bufs=4) as eqp,
            tc.tile_pool(name="ps1", bufs=3, space="PSUM") as ps1,
            tc.tile_pool(name="ps2", bufs=2, space="PSUM") as ps2,
            tc.tile_pool(name="fin", bufs=3) as finp,
        ):
            # Q7 library holding DMAGatherAnt; must precede all gathers
            nc.gpsimd.load_library(library_config.mlp)

            # idx warmup slices first, in their own tiles: the first gathers
            # must depend only on these small DMAs, not on the bulk idx load
            # or the big constant loads
            idx_warm = []
            idx_rest = []
            warm_cols = min(sum(WARM), int(half_len.min())) // 16
            for h in range(n_halves):
                cols = int(half_len[h]) // 16
                wcols = min(warm_cols, cols)
                tw = constp.tile([128, wcols], i16, tag="idxw%d" % h)
                nc.sync.dma_start(tw[:], idx_d[h][:, :wcols])
                idx_warm.append((tw, wcols))
                idx_rest.append(None)

            # --- constants / metadata (weights and biases arrive already
            # scaled by 0.5; x rows arrive scaled by dinv[src])
            iota8 = constp.tile([128, GF, 128], bf16)
            nc.sync.dma_start(
                iota8[:], iota_d[:].rearrange("p (c n) -> p c n", n=128))
            ident = constp.tile([128, 128], bf16)
            nc.sync.dma_start(ident[:], id_d[:])
            wts = {}
            for nm, src_d in (("w1", w1_d), ("w2", w2_d)):
                t = constp.tile([128, 128], bf16, tag=nm)
                nc.sync.dma_start(t[:], src_d[:])
                wts[nm] = t
            bias = {}
            for nm, src_d in (("b1", b1_d), ("b2", b2_d)):
                t = constp.tile([1, 128], bf16, tag=nm)
                nc.sync.dma_start(t[:], src_d[:])
                bias[nm] = t
            rdinv = constp.tile([1, npc], bf16)
            nc.sync.dma_start(rdinv[:], rd_d[:])
            dvo = constp.tile([128, nwin], f32)
            nc.sync.dma_start(dvo[:], dvo_d[:])

            dl = constp.tile([128, nmm], bf16)
            nc.sync.dma_start(dl[:], dl_d[:])

            for h in range(n_halves):
                cols = int(half_len[h]) // 16
                wcols = idx_warm[h][1]
                if wcols < cols:
                    tr = constp.tile([128, cols - wcols], i16,
                                     tag="idxr%d" % h)
                    nc.sync.dma_start(tr[:], idx_d[h][:, wcols:])
                    idx_rest[h] = (tr, wcols)

            # own slab, node-major per window: [128 node, nwin, 128 feat]
            xown = constp.tile([128, nwin, 128], bf16)
            nc.sync.dma_start(
                xown[:, :npc // 128, :],
                xo_d[: (npc // 128) * 128, :].rearrange(
                    "(w p) f -> p w f", p=128))
            if npc % 128:
                nc.sync.dma_start(
                    xown[: npc % 128, npc // 128, :],
                    xo_d[(npc // 128) * 128:, :])

            g_all = constp.tile([128, npc], bf16)

            # one-hot columns: GF fused per DVE is_equal; consumed strictly
            # in column order so a single active group suffices
            eq_cache = [None, None]  # [group id, tile]

            def get_eq(col):
                g = col // GF
                if eq_cache[0] != g:
                    g0 = g * GF
                    gl = min(GF, nmm - g0)
                    eq = eqp.tile([128, GF, 128], bf16, tag="eq")
                    nc.vector.tensor_tensor(
                        out=eq[:, :gl, :], in0=iota8[:, :gl, :],
                        in1=dl[:, g0:g0 + gl, None].to_broadcast([128, gl, 128]),
                        op=OP.is_equal)
                    eq_cache[0] = g
                    eq_cache[1] = eq
                return eq_cache[1]

            # per-half stream state: lazy chunk issuing in window order
            class Stream:
                pass

            streams = []
            for h in range(n_halves):
                s = Stream()
                s.h = h
                s.base = x_d[0:split, :] if h == 0 else x_d[split:n_nodes, :]
                s.chunk_bounds = []
                off = 0
                for L in chunk_sizes[h]:
                    s.chunk_bounds.append((off, L))
                    off += L
                s.blk2chunk = np.repeat(
                    np.arange(len(chunk_sizes[h])),
                    [L // 128 for L in chunk_sizes[h]])
                s.tiles = {}
                streams.append(s)

            ci_global = 0

            def ensure_chunk(s, ci):
                nonlocal ci_global
                if ci in s.tiles:
                    return s.tiles[ci]
                off, L = s.chunk_bounds[ci]
                tw, wcols = idx_warm[s.h]
                if off // 16 < wcols:
                    assert (off + L) // 16 <= wcols, "chunk straddles warm split"
                    it = tw[:, off // 16:(off + L) // 16]
                else:
                    tr, base_cols = idx_rest[s.h]
                    it = tr[:, off // 16 - base_cols:(off + L) // 16 - base_cols]
                xg = xgp.tile([128, chunk // 128, 128], bf16, tag="xg")
                nc.gpsimd.dma_gather(
                    out_ap=xg[:, : L // 128, :],
                    in_ap=s.base,
                    idxs_ap=it,
                    num_idxs=L,
                    num_idxs_reg=L,
                    elem_size=D,
                    single_packet=False,
                    queue_num=ci_global % N_QUEUES,
                )
                ci_global += 1
                s.tiles.clear()
                s.tiles[ci] = xg
                return xg

            # --- output stage: psum = g^T@(W/2) + rdinv*(b/2), then
            # relu with per-partition dst scale dinv[n]; layers averaged
            def emit_phase2(wlo, whi):
                nwg = whi - wlo + 1
                wls = [min(WIN, npc - w * WIN) for w in range(wlo, whi + 1)]
                pps = {}
                for nm_w, nm_b in (("w1", "b1"), ("w2", "b2")):
                    pp = ps2.tile([128, WG * 128], f32, tag="pp")
                    for j, w in enumerate(range(wlo, whi + 1)):
                        wl = wls[j]
                        sl = pp[:wl, j * 128:(j + 1) * 128]
                        nc.tensor.matmul(sl, g_all[:, w * WIN:w * WIN + wl],
                                         wts[nm_w][:], start=True, stop=False)
                        nc.tensor.matmul(sl,
                                         rdinv[:, w * WIN:w * WIN + wl],
                                         bias[nm_b][:], start=False, stop=True)
                    o = finp.tile([128, WG, 128], f32, tag="o" + nm_w)
                    for j, w in enumerate(range(wlo, whi + 1)):
                        nc.scalar.activation(
                            o[:wls[j], j, :],
                            pp[:wls[j], j * 128:(j + 1) * 128], AF.Relu,
                            scale=dvo[:wls[j], w:w + 1])
                    pps[nm_w] = o
                ot = finp.tile([128, WG, 128], f32, tag="ot")
                rows = min(wls)
                otf = ot[:].rearrange("p c n -> p (c n)")
                o1f = pps["w1"][:].rearrange("p c n -> p (c n)")
                o2f = pps["w2"][:].rearrange("p c n -> p (c n)")
                if rows == 128:
                    nc.vector.tensor_tensor(otf[:, :nwg * 128],
                                            o1f[:, :nwg * 128],
                                            o2f[:, :nwg * 128], op=OP.add)
                else:
                    for j in range(nwg):
                        cs = slice(j * 128, j * 128 + 128)
                        nc.vector.tensor_tensor(otf[:wls[j], cs],
                                                o1f[:wls[j], cs],
                                                o2f[:wls[j], cs], op=OP.add)
                for j, w in enumerate(range(wlo, whi + 1)):
                    nc.sync.dma_start(out_d[w * WIN:w * WIN + wls[j], :],
                                      ot[:wls[j], j, :])

            for w in range(nwin):
                wlen = min(WIN, npc - w * WIN)
                pw = ps1.tile([128, 128], f32, tag="pw")
                mms = win_mms[w]
                n_tot = len(mms) + 1
                # self-loop first: x'_own rows -> columns via identity
                nc.tensor.matmul(pw[:, :wlen], xown[:wlen, w, :],
                                 ident[:wlen, :wlen],
                                 start=True, stop=(n_tot == 1))
                for k, (h, blk, col) in enumerate(mms):
                    s = streams[h]
                    ci = int(s.blk2chunk[blk])
                    xg = ensure_chunk(s, ci)
                    bl = blk - s.chunk_bounds[ci][0] // 128
                    eq = get_eq(col)
                    nc.tensor.matmul(
                        pw[:, :wlen],
                        xg[:, bl, :],
                        eq[:, col % GF, :wlen],
                        start=False,
                        stop=(k == n_tot - 2),
                    )
                nc.scalar.activation(g_all[:, w * WIN:w * WIN + wlen],
                                     pw[:, :wlen], AF.Copy)
                if w % WG == WG - 1 or w == nwin - 1:
                    emit_phase2(w - (w % WG), w)

    nc.compile()
    return nc


def make_core_inputs(meta, per_core_inputs, x, W1, b1, W2, b2):
    """Full in_maps for run_bass_kernel_spmd (adds shared tensors).

    x rows are pre-scaled by dinv[src] so gathered rows carry the source
    normalization; weights/biases fold in the 0.5 layer average.
    """
    import ml_dtypes
    bf = ml_dtypes.bfloat16
    dinv = meta["dinv"]
    npc = meta["npc"]
    xs = (np.asarray(x, np.float32) * dinv[:, None]).astype(bf)
    xs = np.ascontiguousarray(xs)
    shared = {
        "x": xs,
        "W1": np.ascontiguousarray((0.5 * np.asarray(W1, np.float32)).astype(bf)),
        "W2": np.ascontiguousarray((0.5 * np.asarray(W2, np.float32)).astype(bf)),
        "b1": (0.5 * np.asarray(b1, np.float32)).astype(bf).reshape(1, D),
        "b2": (0.5 * np.asarray(b2, np.float32)).astype(bf).reshape(1, D),
        "iota": np.ascontiguousarray(np.broadcast_to(
            np.tile(np.arange(128, dtype=np.float32), GF),
            (128, GF * 128)).astype(bf)),
        "ident": np.ascontiguousarray(np.eye(128, dtype=np.float32).astype(bf)),
    }
    maps = []
    for c, ci in enumerate(per_core_inputs):
        m = dict(shared)
        m["x_own"] = np.ascontiguousarray(xs[c * npc:(c + 1) * npc, :])
        m["dstloc"] = np.ascontiguousarray(ci["dstloc"].astype(bf))
        m["dinv_own"] = np.ascontiguousarray(ci["dinv_own"].astype(np.float32))
        m["rdinv"] = np.ascontiguousarray(ci["rdinv"].astype(bf))
        for k, v in ci.items():
            if k.startswith("idx_"):
                m[k] = v
        maps.append(m)
    return maps


# ------------------------------------------------------------------- kernel

def kernel(x, edge_index, W1, b1, W2, b2, _trace=False):
    from concourse.bass_utils import run_bass_kernel_spmd

    x = np.asarray(x)
    n_nodes = x.shape[0]
    meta, pci = host_prep(edge_index, n_nodes, N_CORES)
    nc = build_program(meta)
    in_maps = make_core_inputs(meta, pci, x, W1, b1, W2, b2)
    res = run_bass_kernel_spmd(nc, in_maps, list(range(N_CORES)),
                               trace=_trace)
    out = np.concatenate([res.results[c]["out"] for c in range(N_CORES)],
                         axis=0)
    if _trace:
        return out, res
    return out


# revision 18
# speedup vs baseline: 1.1330x; 1.1224x over previous
"""Two-layer GCN (MultiOrderGraphLayer) Bass kernel for 8 Trainium2 cores.

Math: out = 0.5*(relu(A_hat@x@W1+b1) + relu(A_hat@x@W2+b2)) with
A_hat = D^-1/2 (A+I) D^-1/2.  Both layers share A_hat, so g = A_hat @ x is
computed once and the two small 128x128 matmuls are applied afterwards.

Normalization factorization: norm_e = dinv[src]*dinv[dst].  The host
pre-scales x rows by dinv (x' = dinv[i]*x[i], bf16), so gathered rows
already carry the src factor; the dst factor dinv[n] is applied in the
output stage as a per-partition activation scale (partition = node there).
Self-loops reduce to g_raw[:, n] += x'[n, :], added per window with one
identity matmul instead of gather slots.

Device algorithm (per core, feature-major g_T = [128 feat, nodes]):
  - nodes sharded 8 ways by row; edges partitioned by destination core and
    grouped per 128-node output window into variable-length cells packed
    back-to-back (cell length = max edge count across cores, so the SPMD
    program shape is shared; per-core shortfall is masked via dstloc=-1).
  - sources split into lo (<32768) / hi streams for int16 dma_gather.
  - the stream is consumed in fixed 128-slot blocks; for each (window,
    overlapping block) pair the host emits a dstloc column; slots outside
    the window (straddle or pad) carry -1 and the one-hot masks them out.
  - per block-use: build S[e, n] = (dstloc_e == n) in one DVE is_equal
    (bf16, GF columns fused); accumulate psum += xg^T @ S.
  - output: psum = g_T^T@(0.5W) + rdinv*(0.5b); out = relu(psum * dinv[n])
    (per-partition scale), layers averaged, written node-major.
"""

import math
import numpy as np

N_NODES = 50000
D = 128
N_CORES = 8
SPLIT = 32768  # int16 gather index limit
WIN = 128      # output-window size in nodes (one-hot width / psum free dim)
CHUNK = 4096   # slots per dma_gather instruction (multiple of 128)
N_QUEUES = 4   # SWDGE queues (ucode max); rotation parallelizes drain
GF = 8         # one-hot columns fused per DVE is_equal op
WARM = (1024, 1024, 2048, 2048)  # warmup chunks; keep in sync with idx split


# ---------------------------------------------------------------- host prep

def host_prep(edge_index, n_nodes, n_cores, split=SPLIT, chunk=CHUNK):
    """Edge partitioning by destination core, per-window cells (variable
    length, shared shape across cores), lo/hi source split, block/window
    overlap map, dstloc mask columns.

    All heavy math stays on device; host work is indexing plus the x
    prescale (done in make_core_inputs).
    """
    src = np.asarray(edge_index[0], dtype=np.int64)
    dst = np.asarray(edge_index[1], dtype=np.int64)
    deg = np.bincount(dst, minlength=n_nodes).astype(np.int64) + 1
    dinv = (1.0 / np.sqrt(deg.astype(np.float64))).astype(np.float32)

    npc = n_nodes // n_cores
    assert npc * n_cores == n_nodes
    nwin = math.ceil(npc / WIN)
    n_halves = 2 if n_nodes > split else 1

    per_core = []
    counts = np.zeros((n_cores, n_halves, nwin), np.int64)
    for c in range(n_cores):
        n0 = c * npc
        m = (dst >= n0) & (dst < n0 + npc)
        s, d = src[m], dst[m]
        w = (d - n0) // WIN
        half = (s >= split).astype(np.int64) if n_halves == 2 else np.zeros_like(s)
        key = half * nwin + w
        order = np.argsort(key, kind="stable")
        s, d, key = s[order], d[order], key[order]
        cnt = np.bincount(key, minlength=n_halves * nwin)
        counts[c] = cnt.reshape(n_halves, nwin)
        per_core.append((s, d, cnt))

    # shared cell lengths (max across cores) and packed offsets per half
    clen = counts.max(axis=0)                       # [n_halves, nwin]
    coff = np.zeros_like(clen)
    half_len = np.zeros(n_halves, np.int64)
    for h in range(n_halves):
        coff[h] = np.concatenate([[0], np.cumsum(clen[h])[:-1]])
        half_len[h] = -(-int(clen[h].sum()) // 128) * 128  # pad tail to 128

    # block/window overlap map (compile-time, shared across cores):
    # mm list per window = blocks intersecting [coff, coff+clen)
    mm_cols = []   # global column order: for w: lo block uses, hi block uses
    win_mms = [[] for _ in range(nwin)]
    for w in range(nwin):
        for h in range(n_halves):
            a, b = int(coff[h, w]), int(coff[h, w] + clen[h, w])
            if b == a:
                continue
            for blk in range(a // 128, -(-b // 128)):
                win_mms[w].append((h, blk, len(mm_cols)))
                mm_cols.append((h, blk, w))
    nmm = len(mm_cols)

    # chunk split per half: warmup small, bulk CHUNK, cooldown small
    chunk_sizes = []
    for h in range(n_halves):
        rem, sizes = int(half_len[h]), []
        for wsz in WARM:
            L = min(wsz, rem)
            if L > 0:
                sizes.append(L)
                rem -= L
        while rem > 0:
            L = min(chunk, rem)
            sizes.append(L)
            rem -= L
        if sizes and sizes[-1] == chunk:
            sizes[-1:] = [chunk // 2, chunk // 4, chunk // 4]
        chunk_sizes.append(sizes)

    per_core_inputs = []
    for c in range(n_cores):
        s, d, cnt = per_core[c]
        offs = np.concatenate([[0], np.cumsum(cnt)])
        # slot streams per half: sources (idx) and per-slot local dst
        idx_stream = [np.zeros(int(half_len[h]), np.int64) for h in range(n_halves)]
        dst_stream = [np.full(int(half_len[h]), -1, np.int64) for h in range(n_halves)]
        for h in range(n_halves):
            for w in range(nwin):
                k = h * nwin + w
                a, b = int(offs[k]), int(offs[k + 1])
                o = int(coff[h, w])
                idx_stream[h][o:o + (b - a)] = s[a:b] - h * split
                dst_stream[h][o:o + (b - a)] = d[a:b] - c * npc - w * WIN

        # dstloc mask columns: one [128] column per (window, block) use;
        # slots outside the window's cell get -1
        dl = np.full((128, nmm), -1.0, np.float32)
        for col, (h, blk, w) in enumerate(mm_cols):
            s0, s1 = blk * 128, blk * 128 + 128
            a, b = int(coff[h, w]), int(coff[h, w] + clen[h, w])
            lo, hi = max(s0, a), min(s1, b)
            if hi > lo:
                seg = dst_stream[h][lo:hi].astype(np.float32)
                # mask slots whose dst is outside this window (pad slots
                # carry -1 already; straddle slots belong to w by range)
                dl[lo - s0:hi - s0, col] = seg
        core_in = {"dstloc": np.ascontiguousarray(dl)}

        # own-slab metadata for self-loops + output normalization
        nd = np.arange(npc, dtype=np.int64) + c * npc
        dv = dinv[nd]                                   # [npc]
        pad = nwin * WIN - npc
        dvp = np.concatenate([dv, np.zeros(pad, np.float32)])
        core_in["dinv_own"] = np.ascontiguousarray(
            dvp.reshape(nwin, WIN).T)                   # [128, nwin]
        core_in["rdinv"] = (1.0 / dvp.reshape(1, -1)[:, :npc].clip(1e-30)
                            ).astype(np.float32)        # [1, npc]

        # gather indices: wrapped [16, L/16] per chunk, replicated 8x
        for h in range(n_halves):
            cols, off = [], 0
            for L in chunk_sizes[h]:
                a = idx_stream[h][off:off + L].reshape(-1, 16).T
                cols.append(a)
                off += L
            wrapped = np.concatenate(cols, axis=1).astype(np.int16)
            core_in["idx_h%d" % h] = np.ascontiguousarray(
                np.tile(wrapped, (8, 1)))
        per_core_inputs.append(core_in)

    meta = dict(n_nodes=n_nodes, n_cores=n_cores, npc=npc, nwin=nwin,
                n_halves=n_halves, split=split, half_len=half_len,
                nmm=nmm, win_mms=win_mms, chunk=chunk,
                chunk_sizes=chunk_sizes, dinv=dinv)
    return meta, per_core_inputs


# ------------------------------------------------------------- bass program

def build_program(meta):
    import concourse.bacc as bacc
    import concourse.mybir as mybir
    import concourse.tile as tile
    from concourse import library_config

    f32 = mybir.dt.float32
    bf16 = mybir.dt.bfloat16
    i16 = mybir.dt.int16
    AF = mybir.ActivationFunctionType
    OP = mybir.AluOpType

    n_nodes = meta["n_nodes"]
    npc, nwin = meta["npc"], meta["nwin"]
    n_halves, split = meta["n_halves"], meta["split"]
    half_len = meta["half_len"]
    nmm = meta["nmm"]
    win_mms = meta["win_mms"]
    chunk = meta["chunk"]
    chunk_sizes = meta["chunk_sizes"]
    warm_tot = sum(WARM)

    nc = bacc.Bacc("TRN2", num_swdge_queues=N_QUEUES)

    x_d = nc.declare_dram_parameter("x", [n_nodes, D], bf16, isOutput=False)
    xo_d = nc.declare_dram_parameter("x_own", [npc, D], bf16, isOutput=False)
    dl_d = nc.declare_dram_parameter("dstloc", [128, nmm], bf16, isOutput=False)
    dvo_d = nc.declare_dram_parameter("dinv_own", [128, nwin], f32,
                                      isOutput=False)
    rd_d = nc.declare_dram_parameter("rdinv", [1, npc], bf16, isOutput=False)
    idx_d = [nc.declare_dram_parameter("idx_h%d" % h,
                                       [128, int(half_len[h]) // 16],
                                       i16, isOutput=False)
             for h in range(n_halves)]
    w1_d = nc.declare_dram_parameter("W1", [D, D], bf16, isOutput=False)
    w2_d = nc.declare_dram_parameter("W2", [D, D], bf16, isOutput=False)
    b1_d = nc.declare_dram_parameter("b1", [1, D], bf16, isOutput=False)
    b2_d = nc.declare_dram_parameter("b2", [1, D], bf16, isOutput=False)
    iota_d = nc.declare_dram_parameter("iota", [128, GF * 128], bf16,
                                       isOutput=False)
    id_d = nc.declare_dram_parameter("ident", [128, 128], bf16, isOutput=False)
    out_d = nc.declare_dram_parameter("out", [npc, D], f32, isOutput=True)

    WG = 4  # windows per phase-2 batch (one 512-wide psum bank)

    with tile.TileContext(nc) as tc:
        with (
            tc.tile_pool(name="const", bufs=1) as constp,
            tc.tile_pool(name="xg", bufs=8) as xgp,
            tc.tile_pool(name="eq", bufs=4) as eqp,
            tc.tile_pool(name="ps1", bufs=3, space="PSUM") as ps1,
            tc.tile_pool(name="ps2", bufs=2, space="PSUM") as ps2,
            tc.tile_pool(name="fin", bufs=3) as finp,
        ):
            # Q7 library holding DMAGatherAnt; must precede all gathers
            nc.gpsimd.load_library(library_config.mlp)

            # --- constants / metadata (weights and biases arrive already
            # scaled by 0.5; x rows arrive scaled by dinv[src])
            iota8 = constp.tile([128, GF, 128], bf16)
            nc.sync.dma_start(
                iota8[:], iota_d[:].rearrange("p (c n) -> p c n", n=128))
            ident = constp.tile([128, 128], bf16)
            nc.sync.dma_start(ident[:], id_d[:])
            wts = {}
            for nm, src_d in (("w1", w1_d), ("w2", w2_d)):
                t = constp.tile([128, 128], bf16, tag=nm)
                nc.sync.dma_start(t[:], src_d[:])
                wts[nm] = t
            bias = {}
            for nm, src_d in (("b1", b1_d), ("b2", b2_d)):
                t = constp.tile([1, 128], bf16, tag=nm)
                nc.sync.dma_start(t[:], src_d[:])
                bias[nm] = t
            rdinv = constp.tile([1, npc], bf16)
            nc.sync.dma_start(rdinv[:], rd_d[:])
            dvo = constp.tile([128, nwin], f32)
            nc.sync.dma_start(dvo[:], dvo_d[:])

            dl = constp.tile([128, nmm], bf16)
            nc.sync.dma_start(dl[:], dl_d[:])

            # own slab, node-major per window: [128 node, nwin, 128 feat]
            xown = constp.tile([128, nwin, 128], bf16)
            nc.sync.dma_start(
                xown[:, :npc // 128, :],
                xo_d[: (npc // 128) * 128, :].rearrange(
                    "(w p) f -> p w f", p=128))
            if npc % 128:
                nc.sync.dma_start(
                    xown[: npc % 128, npc // 128, :],
                    xo_d[(npc // 128) * 128:, :])

            g_all = constp.tile([128, npc], bf16)

            # idx streams: split the preload so the warmup chunks' indices
            # land quickly and the first gather starts early
            idx_all = []
            for h in range(n_halves):
                cols = int(half_len[h]) // 16
                wcols = min(warm_tot // 16, cols)
                t = constp.tile([128, cols], i16, tag="idx%d" % h)
                nc.sync.dma_start(t[:, :wcols], idx_d[h][:, :wcols])
                if wcols < cols:
                    nc.sync.dma_start(t[:, wcols:], idx_d[h][:, wcols:])
                idx_all.append(t)

            # one-hot columns: GF fused per DVE is_equal; consumed strictly
            # in column order so a single active group suffices
            eq_cache = [None, None]  # [group id, tile]

            def get_eq(col):
                g = col // GF
                if eq_cache[0] != g:
                    g0 = g * GF
                    gl = min(GF, nmm - g0)
                    eq = eqp.tile([128, GF, 128], bf16, tag="eq")
                    nc.vector.tensor_tensor(
                        out=eq[:, :gl, :], in0=iota8[:, :gl, :],
                        in1=dl[:, g0:g0 + gl, None].to_broadcast([128, gl, 128]),
                        op=OP.is_equal)
                    eq_cache[0] = g
                    eq_cache[1] = eq
                return eq_cache[1]

            # per-half stream state: lazy chunk issuing in window order
            class Stream:
                pass

            streams = []
            for h in range(n_halves):
                s = Stream()
                s.h = h
                s.base = x_d[0:split, :] if h == 0 else x_d[split:n_nodes, :]
                s.chunk_bounds = []
                off = 0
                for L in chunk_sizes[h]:
                    s.chunk_bounds.append((off, L))
                    off += L
                s.blk2chunk = np.repeat(
                    np.arange(len(chunk_sizes[h])),
                    [L // 128 for L in chunk_sizes[h]])
                s.tiles = {}
                streams.append(s)

            ci_global = 0

            def ensure_chunk(s, ci):
                nonlocal ci_global
                if ci in s.tiles:
                    return s.tiles[ci]
                off, L = s.chunk_bounds[ci]
                xg = xgp.tile([128, chunk // 128, 128], bf16, tag="xg")
                nc.gpsimd.dma_gather(
                    out_ap=xg[:, : L // 128, :],
                    in_ap=s.base,
                    idxs_ap=idx_all[s.h][:, off // 16:(off + L) // 16],
                    num_idxs=L,
                    num_idxs_reg=L,
                    elem_size=D,
                    single_packet=False,
                    queue_num=ci_global % N_QUEUES,
                )
                ci_global += 1
                s.tiles.clear()
                s.tiles[ci] = xg
                return xg

            # --- output stage: psum = g^T@(W/2) + rdinv*(b/2), then
            # relu with per-partition dst scale dinv[n]; layers averaged
            def emit_phase2(wlo, whi):
                nwg = whi - wlo + 1
                wls = [min(WIN, npc - w * WIN) for w in range(wlo, whi + 1)]
                pps = {}
                for nm_w, nm_b in (("w1", "b1"), ("w2", "b2")):
                    pp = ps2.tile([128, WG * 128], f32, tag="pp")
                    for j, w in enumerate(range(wlo, whi + 1)):
                        wl = wls[j]
                        sl = pp[:wl, j * 128:(j + 1) * 128]
                        nc.tensor.matmul(sl, g_all[:, w * WIN:w * WIN + wl],
                                         wts[nm_w][:], start=True, stop=False)
                        nc.tensor.matmul(sl,
                                         rdinv[:, w * WIN:w * WIN + wl],
                                         bias[nm_b][:], start=False, stop=True)
                    o = finp.tile([128, WG, 128], f32, tag="o" + nm_w)
                    for j, w in enumerate(range(wlo, whi + 1)):
                        nc.scalar.activation(
                            o[:wls[j], j, :],
                            pp[:wls[j], j * 128:(j + 1) * 128], AF.Relu,
                            scale=dvo[:wls[j], w:w + 1])
                    pps[nm_w] = o
                ot = finp.tile([128, WG, 128], f32, tag="ot")
                rows = min(wls)
                otf = ot[:].rearrange("p c n -> p (c n)")
                o1f = pps["w1"][:].rearrange("p c n -> p (c n)")
                o2f = pps["w2"][:].rearrange("p c n -> p (c n)")
                if rows == 128:
                    nc.vector.tensor_tensor(otf[:, :nwg * 128],
                                            o1f[:, :nwg * 128],
                                            o2f[:, :nwg * 128], op=OP.add)
                else:
                    for j in range(nwg):
                        cs = slice(j * 128, j * 128 + 128)
                        nc.vector.tensor_tensor(otf[:wls[j], cs],
                                                o1f[:wls[j], cs],
                                                o2f[:wls[j], cs], op=OP.add)
                for j, w in enumerate(range(wlo, whi + 1)):
                    nc.sync.dma_start(out_d[w * WIN:w * WIN + wls[j], :],
                                      ot[:wls[j], j, :])

            for w in range(nwin):
                wlen = min(WIN, npc - w * WIN)
                pw = ps1.tile([128, 128], f32, tag="pw")
                mms = win_mms[w]
                n_tot = len(mms) + 1
                # self-loop first: x'_own rows -> columns via identity
                nc.tensor.matmul(pw[:, :wlen], xown[:wlen, w, :],
                                 ident[:wlen, :wlen],
                                 start=True, stop=(n_tot == 1))
                for k, (h, blk, col) in enumerate(mms):
                    s = streams[h]
                    ci = int(s.blk2chunk[blk])
                    xg = ensure_chunk(s, ci)
                    bl = blk - s.chunk_bounds[ci][0] // 128
                    eq = get_eq(col)
                    nc.tensor.matmul(
                        pw[:, :wlen],
                        xg[:, bl, :],
                        eq[:, col % GF, :wlen],
                        start=False,
                        stop=(k == n_tot - 2),
                    )
                nc.scalar.activation(g_all[:, w * WIN:w * WIN + wlen],
                                     pw[:, :wlen], AF.Copy)
                if w % WG == WG - 1 or w == nwin - 1:
                    emit_phase2(w - (w % WG), w)

    nc.compile()
    return nc


def make_core_inputs(meta, per_core_inputs, x, W1, b1, W2, b2):
    """Full in_maps for run_bass_kernel_spmd (adds shared tensors).

    x rows are pre-scaled by dinv[src] so gathered rows carry the source
    normalization; weights/biases fold in the 0.5 layer average.
    """
    import ml_dtypes
    bf = ml_dtypes.bfloat16
    dinv = meta["dinv"]
    npc = meta["npc"]
    xs = (np.asarray(x, np.float32) * dinv[:, None]).astype(bf)
    xs = np.ascontiguousarray(xs)
    shared = {
        "x": xs,
        "W1": np.ascontiguousarray((0.5 * np.asarray(W1, np.float32)).astype(bf)),
        "W2": np.ascontiguousarray((0.5 * np.asarray(W2, np.float32)).astype(bf)),
        "b1": (0.5 * np.asarray(b1, np.float32)).astype(bf).reshape(1, D),
        "b2": (0.5 * np.asarray(b2, np.float32)).astype(bf).reshape(1, D),
        "iota": np.ascontiguousarray(np.broadcast_to(
            np.tile(np.arange(128, dtype=np.float32), GF),
            (128, GF * 128)).astype(bf)),
        "ident": np.ascontiguousarray(np.eye(128, dtype=np.float32).astype(bf)),
    }
    maps = []
    for c, ci in enumerate(per_core_inputs):
        m = dict(shared)
        m["x_own"] = np.ascontiguousarray(xs[c * npc:(c + 1) * npc, :])
        m["dstloc"] = np.ascontiguousarray(ci["dstloc"].astype(bf))
        m["dinv_own"] = np.ascontiguousarray(ci["dinv_own"].astype(np.float32))
        m["rdinv"] = np.ascontiguousarray(ci["rdinv"].astype(bf))
        for k, v in ci.items():
            if k.startswith("idx_"):
                m[k] = v
        maps.append(m)
    return maps


# ------------------------------------------------------------------- kernel

def kernel(x, edge_index, W1, b1, W2, b2, _trace=False):
    from concourse.bass_utils import run_bass_kernel_spmd

    x = np.asarray(x)
    n_nodes = x.shape[0]
    meta, pci = host_prep(edge_index, n_nodes, N_CORES)
    nc = build_program(meta)
    in_maps = make_core_inputs(meta, pci, x, W1, b1, W2, b2)
    res = run_bass_kernel_spmd(nc, in_maps, list(range(N_CORES)),
                               trace=_trace)
    out = np.concatenate([res.results[c]["out"] for c in range(N_CORES)],
                         axis=0)
    if _trace:
        return out, res
    return out


# revision 19
# speedup vs baseline: 1.2032x; 1.0620x over previous
"""Two-layer GCN (MultiOrderGraphLayer) Bass kernel for 8 Trainium2 cores.

Math: out = 0.5*(relu(A_hat@x@W1+b1) + relu(A_hat@x@W2+b2)) with
A_hat = D^-1/2 (A+I) D^-1/2.  Both layers share A_hat, so g = A_hat @ x is
computed once and the two small 128x128 matmuls are applied afterwards.

Normalization factorization: norm_e = dinv[src]*dinv[dst].  The host
pre-scales x rows by dinv (x' = dinv[i]*x[i], bf16), so gathered rows
already carry the src factor; the dst factor dinv[n] is applied in the
output stage as a per-partition activation scale (partition = node there).
Self-loops reduce to g_raw[:, n] += x'[n, :], added per window with one
identity matmul instead of gather slots.

Device algorithm (per core, feature-major g_T = [128 feat, nodes]):
  - nodes sharded 8 ways by row; edges partitioned by destination core and
    grouped per 128-node output window into variable-length cells packed
    back-to-back (cell length = max edge count across cores, so the SPMD
    program shape is shared; per-core shortfall is masked via dstloc=-1).
  - sources split into lo (<32768) / hi streams for int16 dma_gather.
  - the stream is consumed in fixed 128-slot blocks; for each (window,
    overlapping block) pair the host emits a dstloc column; slots outside
    the window (straddle or pad) carry -1 and the one-hot masks them out.
  - per block-use: build S[e, n] = (dstloc_e == n) in one DVE is_equal
    (bf16, GF columns fused); accumulate psum += xg^T @ S.
  - output: psum = g_T^T@(0.5W) + rdinv*(0.5b); out = relu(psum * dinv[n])
    (per-partition scale), layers averaged, written node-major.
"""

import math
import numpy as np

N_NODES = 50000
D = 128
N_CORES = 8
SPLIT = 32768  # int16 gather index limit
WIN = 128      # output-window size in nodes (one-hot width / psum free dim)
CHUNK = 2048   # slots per dma_gather instruction (multiple of 128); kept
               # small so two in-flight gathers per SWDGE queue fit the
               # ~16KB/partition descriptor ring without blocking emission
N_QUEUES = 4   # SWDGE queues (ucode max); rotation parallelizes drain
GF = 8         # one-hot columns fused per DVE is_equal op
WARM = (1024, 1024, 2048, 2048)  # warmup chunks; keep in sync with idx split


# ---------------------------------------------------------------- host prep

def host_prep(edge_index, n_nodes, n_cores, split=SPLIT, chunk=CHUNK):
    """Edge partitioning by destination core, per-window cells (variable
    length, shared shape across cores), lo/hi source split, block/window
    overlap map, dstloc mask columns.

    All heavy math stays on device; host work is indexing plus the x
    prescale (done in make_core_inputs).
    """
    src = np.asarray(edge_index[0], dtype=np.int64)
    dst = np.asarray(edge_index[1], dtype=np.int64)
    deg = np.bincount(dst, minlength=n_nodes).astype(np.int64) + 1
    dinv = (1.0 / np.sqrt(deg.astype(np.float64))).astype(np.float32)

    npc = n_nodes // n_cores
    assert npc * n_cores == n_nodes
    nwin = math.ceil(npc / WIN)
    n_halves = 2 if n_nodes > split else 1

    per_core = []
    counts = np.zeros((n_cores, n_halves, nwin), np.int64)
    for c in range(n_cores):
        n0 = c * npc
        m = (dst >= n0) & (dst < n0 + npc)
        s, d = src[m], dst[m]
        w = (d - n0) // WIN
        half = (s >= split).astype(np.int64) if n_halves == 2 else np.zeros_like(s)
        key = half * nwin + w
        order = np.argsort(key, kind="stable")
        s, d, key = s[order], d[order], key[order]
        cnt = np.bincount(key, minlength=n_halves * nwin)
        counts[c] = cnt.reshape(n_halves, nwin)
        per_core.append((s, d, cnt))

    # shared cell lengths (max across cores) and packed offsets per half
    clen = counts.max(axis=0)                       # [n_halves, nwin]
    coff = np.zeros_like(clen)
    half_len = np.zeros(n_halves, np.int64)
    for h in range(n_halves):
        coff[h] = np.concatenate([[0], np.cumsum(clen[h])[:-1]])
        half_len[h] = -(-int(clen[h].sum()) // 128) * 128  # pad tail to 128

    # block/window overlap map (compile-time, shared across cores):
    # mm list per window = blocks intersecting [coff, coff+clen)
    mm_cols = []   # global column order: for w: lo block uses, hi block uses
    win_mms = [[] for _ in range(nwin)]
    for w in range(nwin):
        for h in range(n_halves):
            a, b = int(coff[h, w]), int(coff[h, w] + clen[h, w])
            if b == a:
                continue
            for blk in range(a // 128, -(-b // 128)):
                win_mms[w].append((h, blk, len(mm_cols)))
                mm_cols.append((h, blk, w))
    nmm = len(mm_cols)

    # chunk split per half: warmup small, bulk CHUNK, cooldown small
    chunk_sizes = []
    for h in range(n_halves):
        rem, sizes = int(half_len[h]), []
        for wsz in WARM:
            L = min(wsz, rem)
            if L > 0:
                sizes.append(L)
                rem -= L
        while rem > 0:
            L = min(chunk, rem)
            sizes.append(L)
            rem -= L
        if sizes and sizes[-1] == chunk:
            sizes[-1:] = [chunk // 2, chunk // 4, chunk // 4]
        chunk_sizes.append(sizes)

    per_core_inputs = []
    for c in range(n_cores):
        s, d, cnt = per_core[c]
        offs = np.concatenate([[0], np.cumsum(cnt)])
        # slot streams per half: sources (idx) and per-slot local dst
        idx_stream = [np.zeros(int(half_len[h]), np.int64) for h in range(n_halves)]
        dst_stream = [np.full(int(half_len[h]), -1, np.int64) for h in range(n_halves)]
        for h in range(n_halves):
            for w in range(nwin):
                k = h * nwin + w
                a, b = int(offs[k]), int(offs[k + 1])
                o = int(coff[h, w])
                idx_stream[h][o:o + (b - a)] = s[a:b] - h * split
                dst_stream[h][o:o + (b - a)] = d[a:b] - c * npc - w * WIN

        # dstloc mask columns: one [128] column per (window, block) use;
        # slots outside the window's cell get -1
        dl = np.full((128, nmm), -1.0, np.float32)
        for col, (h, blk, w) in enumerate(mm_cols):
            s0, s1 = blk * 128, blk * 128 + 128
            a, b = int(coff[h, w]), int(coff[h, w] + clen[h, w])
            lo, hi = max(s0, a), min(s1, b)
            if hi > lo:
                seg = dst_stream[h][lo:hi].astype(np.float32)
                # mask slots whose dst is outside this window (pad slots
                # carry -1 already; straddle slots belong to w by range)
                dl[lo - s0:hi - s0, col] = seg
        core_in = {"dstloc": np.ascontiguousarray(dl)}

        # own-slab metadata for self-loops + output normalization
        nd = np.arange(npc, dtype=np.int64) + c * npc
        dv = dinv[nd]                                   # [npc]
        pad = nwin * WIN - npc
        dvp = np.concatenate([dv, np.zeros(pad, np.float32)])
        core_in["dinv_own"] = np.ascontiguousarray(
            dvp.reshape(nwin, WIN).T)                   # [128, nwin]
        core_in["rdinv"] = (1.0 / dvp.reshape(1, -1)[:, :npc].clip(1e-30)
                            ).astype(np.float32)        # [1, npc]

        # gather indices: wrapped [16, L/16] per chunk, replicated 8x
        for h in range(n_halves):
            cols, off = [], 0
            for L in chunk_sizes[h]:
                a = idx_stream[h][off:off + L].reshape(-1, 16).T
                cols.append(a)
                off += L
            wrapped = np.concatenate(cols, axis=1).astype(np.int16)
            core_in["idx_h%d" % h] = np.ascontiguousarray(
                np.tile(wrapped, (8, 1)))
        per_core_inputs.append(core_in)

    meta = dict(n_nodes=n_nodes, n_cores=n_cores, npc=npc, nwin=nwin,
                n_halves=n_halves, split=split, half_len=half_len,
                nmm=nmm, win_mms=win_mms, chunk=chunk,
                chunk_sizes=chunk_sizes, dinv=dinv)
    return meta, per_core_inputs


# ------------------------------------------------------------- bass program

def build_program(meta):
    import concourse.bacc as bacc
    import concourse.mybir as mybir
    import concourse.tile as tile
    from concourse import library_config

    f32 = mybir.dt.float32
    bf16 = mybir.dt.bfloat16
    i16 = mybir.dt.int16
    AF = mybir.ActivationFunctionType
    OP = mybir.AluOpType

    n_nodes = meta["n_nodes"]
    npc, nwin = meta["npc"], meta["nwin"]
    n_halves, split = meta["n_halves"], meta["split"]
    half_len = meta["half_len"]
    nmm = meta["nmm"]
    win_mms = meta["win_mms"]
    chunk = meta["chunk"]
    chunk_sizes = meta["chunk_sizes"]
    warm_tot = sum(WARM)

    nc = bacc.Bacc("TRN2", num_swdge_queues=N_QUEUES)

    x_d = nc.declare_dram_parameter("x", [n_nodes, D], bf16, isOutput=False)
    xo_d = nc.declare_dram_parameter("x_own", [npc, D], bf16, isOutput=False)
    dl_d = nc.declare_dram_parameter("dstloc", [128, nmm], bf16, isOutput=False)
    dvo_d = nc.declare_dram_parameter("dinv_own", [128, nwin], f32,
                                      isOutput=False)
    rd_d = nc.declare_dram_parameter("rdinv", [1, npc], bf16, isOutput=False)
    idx_d = [nc.declare_dram_parameter("idx_h%d" % h,
                                       [128, int(half_len[h]) // 16],
                                       i16, isOutput=False)
             for h in range(n_halves)]
    w1_d = nc.declare_dram_parameter("W1", [D, D], bf16, isOutput=False)
    w2_d = nc.declare_dram_parameter("W2", [D, D], bf16, isOutput=False)
    b1_d = nc.declare_dram_parameter("b1", [1, D], bf16, isOutput=False)
    b2_d = nc.declare_dram_parameter("b2", [1, D], bf16, isOutput=False)
    iota_d = nc.declare_dram_parameter("iota", [128, GF * 128], bf16,
                                       isOutput=False)
    id_d = nc.declare_dram_parameter("ident", [128, 128], bf16, isOutput=False)
    out_d = nc.declare_dram_parameter("out", [npc, D], f32, isOutput=True)

    WG = 4  # windows per phase-2 batch (one 512-wide psum bank)

    with tile.TileContext(nc) as tc:
        with (
            tc.tile_pool(name="const", bufs=1) as constp,
            tc.tile_pool(name="xg", bufs=8) as xgp,
            tc.tile_pool(name="eq", bufs=4) as eqp,
            tc.tile_pool(name="ps1", bufs=3, space="PSUM") as ps1,
            tc.tile_pool(name="ps2", bufs=2, space="PSUM") as ps2,
            tc.tile_pool(name="fin", bufs=3) as finp,
        ):
            # Q7 library holding DMAGatherAnt; must precede all gathers
            nc.gpsimd.load_library(library_config.mlp)

            # --- constants / metadata (weights and biases arrive already
            # scaled by 0.5; x rows arrive scaled by dinv[src])
            iota8 = constp.tile([128, GF, 128], bf16)
            nc.sync.dma_start(
                iota8[:], iota_d[:].rearrange("p (c n) -> p c n", n=128))
            ident = constp.tile([128, 128], bf16)
            nc.sync.dma_start(ident[:], id_d[:])
            wts = {}
            for nm, src_d in (("w1", w1_d), ("w2", w2_d)):
                t = constp.tile([128, 128], bf16, tag=nm)
                nc.sync.dma_start(t[:], src_d[:])
                wts[nm] = t
            bias = {}
            for nm, src_d in (("b1", b1_d), ("b2", b2_d)):
                t = constp.tile([1, 128], bf16, tag=nm)
                nc.sync.dma_start(t[:], src_d[:])
                bias[nm] = t
            rdinv = constp.tile([1, npc], bf16)
            nc.sync.dma_start(rdinv[:], rd_d[:])
            dvo = constp.tile([128, nwin], f32)
            nc.sync.dma_start(dvo[:], dvo_d[:])

            dl = constp.tile([128, nmm], bf16)
            nc.sync.dma_start(dl[:], dl_d[:])

            # own slab, node-major per window: [128 node, nwin, 128 feat]
            xown = constp.tile([128, nwin, 128], bf16)
            nc.sync.dma_start(
                xown[:, :npc // 128, :],
                xo_d[: (npc // 128) * 128, :].rearrange(
                    "(w p) f -> p w f", p=128))
            if npc % 128:
                nc.sync.dma_start(
                    xown[: npc % 128, npc // 128, :],
                    xo_d[(npc // 128) * 128:, :])

            g_all = constp.tile([128, npc], bf16)

            # idx streams: split the preload so the warmup chunks' indices
            # land quickly and the first gather starts early
            idx_all = []
            for h in range(n_halves):
                cols = int(half_len[h]) // 16
                wcols = min(warm_tot // 16, cols)
                t = constp.tile([128, cols], i16, tag="idx%d" % h)
                nc.sync.dma_start(t[:, :wcols], idx_d[h][:, :wcols])
                if wcols < cols:
                    nc.sync.dma_start(t[:, wcols:], idx_d[h][:, wcols:])
                idx_all.append(t)

            # one-hot columns: GF fused per DVE is_equal; consumed strictly
            # in column order so a single active group suffices
            eq_cache = [None, None]  # [group id, tile]

            def get_eq(col):
                g = col // GF
                if eq_cache[0] != g:
                    g0 = g * GF
                    gl = min(GF, nmm - g0)
                    eq = eqp.tile([128, GF, 128], bf16, tag="eq")
                    nc.vector.tensor_tensor(
                        out=eq[:, :gl, :], in0=iota8[:, :gl, :],
                        in1=dl[:, g0:g0 + gl, None].to_broadcast([128, gl, 128]),
                        op=OP.is_equal)
                    eq_cache[0] = g
                    eq_cache[1] = eq
                return eq_cache[1]

            # per-half stream state: lazy chunk issuing in window order
            class Stream:
                pass

            streams = []
            for h in range(n_halves):
                s = Stream()
                s.h = h
                s.base = x_d[0:split, :] if h == 0 else x_d[split:n_nodes, :]
                s.chunk_bounds = []
                off = 0
                for L in chunk_sizes[h]:
                    s.chunk_bounds.append((off, L))
                    off += L
                s.blk2chunk = np.repeat(
                    np.arange(len(chunk_sizes[h])),
                    [L // 128 for L in chunk_sizes[h]])
                s.tiles = {}
                streams.append(s)

            ci_global = 0

            def ensure_chunk(s, ci):
                nonlocal ci_global
                if ci in s.tiles:
                    return s.tiles[ci]
                off, L = s.chunk_bounds[ci]
                xg = xgp.tile([128, chunk // 128, 128], bf16, tag="xg")
                nc.gpsimd.dma_gather(
                    out_ap=xg[:, : L // 128, :],
                    in_ap=s.base,
                    idxs_ap=idx_all[s.h][:, off // 16:(off + L) // 16],
                    num_idxs=L,
                    num_idxs_reg=L,
                    elem_size=D,
                    single_packet=False,
                    queue_num=ci_global % N_QUEUES,
                )
                ci_global += 1
                s.tiles.clear()
                s.tiles[ci] = xg
                return xg

            # --- output stage: psum = g^T@(W/2) + rdinv*(b/2), then
            # relu with per-partition dst scale dinv[n]; layers averaged
            def emit_phase2(wlo, whi):
                nwg = whi - wlo + 1
                wls = [min(WIN, npc - w * WIN) for w in range(wlo, whi + 1)]
                pps = {}
                for nm_w, nm_b in (("w1", "b1"), ("w2", "b2")):
                    pp = ps2.tile([128, WG * 128], f32, tag="pp")
                    for j, w in enumerate(range(wlo, whi + 1)):
                        wl = wls[j]
                        sl = pp[:wl, j * 128:(j + 1) * 128]
                        nc.tensor.matmul(sl, g_all[:, w * WIN:w * WIN + wl],
                                         wts[nm_w][:], start=True, stop=False)
                        nc.tensor.matmul(sl,
                                         rdinv[:, w * WIN:w * WIN + wl],
                                         bias[nm_b][:], start=False, stop=True)
                    o = finp.tile([128, WG, 128], f32, tag="o" + nm_w)
                    for j, w in enumerate(range(wlo, whi + 1)):
                        nc.scalar.activation(
                            o[:wls[j], j, :],
                            pp[:wls[j], j * 128:(j + 1) * 128], AF.Relu,
                            scale=dvo[:wls[j], w:w + 1])
                    pps[nm_w] = o
                ot = finp.tile([128, WG, 128], f32, tag="ot")
                rows = min(wls)
                otf = ot[:].rearrange("p c n -> p (c n)")
                o1f = pps["w1"][:].rearrange("p c n -> p (c n)")
                o2f = pps["w2"][:].rearrange("p c n -> p (c n)")
                if rows == 128:
                    nc.vector.tensor_tensor(otf[:, :nwg * 128],
                                            o1f[:, :nwg * 128],
                                            o2f[:, :nwg * 128], op=OP.add)
                else:
                    for j in range(nwg):
                        cs = slice(j * 128, j * 128 + 128)
                        nc.vector.tensor_tensor(otf[:wls[j], cs],
                                                o1f[:wls[j], cs],
                                                o2f[:wls[j], cs], op=OP.add)
                for j, w in enumerate(range(wlo, whi + 1)):
                    nc.sync.dma_start(out_d[w * WIN:w * WIN + wls[j], :],
                                      ot[:wls[j], j, :])

            for w in range(nwin):
                wlen = min(WIN, npc - w * WIN)
                pw = ps1.tile([128, 128], f32, tag="pw")
                mms = win_mms[w]
                n_tot = len(mms) + 1
                # self-loop first: x'_own rows -> columns via identity
                nc.tensor.matmul(pw[:, :wlen], xown[:wlen, w, :],
                                 ident[:wlen, :wlen],
                                 start=True, stop=(n_tot == 1))
                for k, (h, blk, col) in enumerate(mms):
                    s = streams[h]
                    ci = int(s.blk2chunk[blk])
                    xg = ensure_chunk(s, ci)
                    bl = blk - s.chunk_bounds[ci][0] // 128
                    eq = get_eq(col)
                    nc.tensor.matmul(
                        pw[:, :wlen],
                        xg[:, bl, :],
                        eq[:, col % GF, :wlen],
                        start=False,
                        stop=(k == n_tot - 2),
                    )
                nc.scalar.activation(g_all[:, w * WIN:w * WIN + wlen],
                                     pw[:, :wlen], AF.Copy)
                if w % WG == WG - 1 or w == nwin - 1:
                    emit_phase2(w - (w % WG), w)

    nc.compile()
    return nc


def make_core_inputs(meta, per_core_inputs, x, W1, b1, W2, b2):
    """Full in_maps for run_bass_kernel_spmd (adds shared tensors).

    x rows are pre-scaled by dinv[src] so gathered rows carry the source
    normalization; weights/biases fold in the 0.5 layer average.
    """
    import ml_dtypes
    bf = ml_dtypes.bfloat16
    dinv = meta["dinv"]
    npc = meta["npc"]
    xs = (np.asarray(x, np.float32) * dinv[:, None]).astype(bf)
    xs = np.ascontiguousarray(xs)
    shared = {
        "x": xs,
        "W1": np.ascontiguousarray((0.5 * np.asarray(W1, np.float32)).astype(bf)),
        "W2": np.ascontiguousarray((0.5 * np.asarray(W2, np.float32)).astype(bf)),
        "b1": (0.5 * np.asarray(b1, np.float32)).astype(bf).reshape(1, D),
        "b2": (0.5 * np.asarray(b2, np.float32)).astype(bf).reshape(1, D),
        "iota": np.ascontiguousarray(np.broadcast_to(
            np.tile(np.arange(128, dtype=np.float32), GF),
            (128, GF * 128)).astype(bf)),
        "ident": np.ascontiguousarray(np.eye(128, dtype=np.float32).astype(bf)),
    }
    maps = []
    for c, ci in enumerate(per_core_inputs):
        m = dict(shared)
        m["x_own"] = np.ascontiguousarray(xs[c * npc:(c + 1) * npc, :])
        m["dstloc"] = np.ascontiguousarray(ci["dstloc"].astype(bf))
        m["dinv_own"] = np.ascontiguousarray(ci["dinv_own"].astype(np.float32))
        m["rdinv"] = np.ascontiguousarray(ci["rdinv"].astype(bf))
        for k, v in ci.items():
            if k.startswith("idx_"):
                m[k] = v
        maps.append(m)
    return maps


# ------------------------------------------------------------------- kernel

def kernel(x, edge_index, W1, b1, W2, b2, _trace=False):
    from concourse.bass_utils import run_bass_kernel_spmd

    x = np.asarray(x)
    n_nodes = x.shape[0]
    meta, pci = host_prep(edge_index, n_nodes, N_CORES)
    nc = build_program(meta)
    in_maps = make_core_inputs(meta, pci, x, W1, b1, W2, b2)
    res = run_bass_kernel_spmd(nc, in_maps, list(range(N_CORES)),
                               trace=_trace)
    out = np.concatenate([res.results[c]["out"] for c in range(N_CORES)],
                         axis=0)
    if _trace:
        return out, res
    return out


# revision 20
# speedup vs baseline: 1.2122x; 1.0075x over previous
"""Two-layer GCN (MultiOrderGraphLayer) Bass kernel for 8 Trainium2 cores.

Math: out = 0.5*(relu(A_hat@x@W1+b1) + relu(A_hat@x@W2+b2)) with
A_hat = D^-1/2 (A+I) D^-1/2.  Both layers share A_hat, so g = A_hat @ x is
computed once and the two small 128x128 matmuls are applied afterwards.

Normalization factorization: norm_e = dinv[src]*dinv[dst].  The host
pre-scales x rows by dinv (x' = dinv[i]*x[i], bf16), so gathered rows
already carry the src factor; the dst factor dinv[n] is applied in the
output stage as a per-partition activation scale (partition = node there).
Self-loops reduce to g_raw[:, n] += x'[n, :], added per window with one
identity matmul instead of gather slots.

Device algorithm (per core, feature-major g_T = [128 feat, nodes]):
  - nodes sharded 8 ways by row; edges partitioned by destination core and
    grouped per 128-node output window into variable-length cells packed
    back-to-back (cell length = max edge count across cores, so the SPMD
    program shape is shared; per-core shortfall is masked via dstloc=-1).
  - sources split into lo (<32768) / hi streams for int16 dma_gather.
  - the stream is consumed in fixed 128-slot blocks; for each (window,
    overlapping block) pair the host emits a dstloc column; slots outside
    the window (straddle or pad) carry -1 and the one-hot masks them out.
  - per block-use: build S[e, n] = (dstloc_e == n) in one DVE is_equal
    (bf16, GF columns fused); accumulate psum += xg^T @ S.
  - output: psum = g_T^T@(0.5W) + rdinv*(0.5b); out = relu(psum * dinv[n])
    (per-partition scale), layers averaged, written node-major.
"""

import math
import numpy as np

N_NODES = 50000
D = 128
N_CORES = 8
SPLIT = 32768  # int16 gather index limit
WIN = 128      # output-window size in nodes (one-hot width / psum free dim)
CHUNK = 2048   # slots per dma_gather instruction (multiple of 128); kept
               # small so two in-flight gathers per SWDGE queue fit the
               # ~16KB/partition descriptor ring without blocking emission
N_QUEUES = 4   # SWDGE queues (ucode max); rotation parallelizes drain
GF = 8         # one-hot columns fused per DVE is_equal op
WARM = (1024, 1024, 2048, 2048)  # warmup chunks; keep in sync with idx split


# ---------------------------------------------------------------- host prep

def host_prep(edge_index, n_nodes, n_cores, split=SPLIT, chunk=CHUNK):
    """Edge partitioning by destination core, per-window cells (variable
    length, shared shape across cores), lo/hi source split, block/window
    overlap map, dstloc mask columns.

    All heavy math stays on device; host work is indexing plus the x
    prescale (done in make_core_inputs).
    """
    src = np.asarray(edge_index[0], dtype=np.int64)
    dst = np.asarray(edge_index[1], dtype=np.int64)
    deg = np.bincount(dst, minlength=n_nodes).astype(np.int64) + 1
    dinv = (1.0 / np.sqrt(deg.astype(np.float64))).astype(np.float32)

    npc = n_nodes // n_cores
    assert npc * n_cores == n_nodes
    nwin = math.ceil(npc / WIN)
    n_halves = 2 if n_nodes > split else 1

    per_core = []
    counts = np.zeros((n_cores, n_halves, nwin), np.int64)
    for c in range(n_cores):
        n0 = c * npc
        m = (dst >= n0) & (dst < n0 + npc)
        s, d = src[m], dst[m]
        w = (d - n0) // WIN
        half = (s >= split).astype(np.int64) if n_halves == 2 else np.zeros_like(s)
        key = half * nwin + w
        order = np.argsort(key, kind="stable")
        s, d, key = s[order], d[order], key[order]
        cnt = np.bincount(key, minlength=n_halves * nwin)
        counts[c] = cnt.reshape(n_halves, nwin)
        per_core.append((s, d, cnt))

    # shared cell lengths (max across cores) and packed offsets per half
    clen = counts.max(axis=0)                       # [n_halves, nwin]
    coff = np.zeros_like(clen)
    half_len = np.zeros(n_halves, np.int64)
    for h in range(n_halves):
        coff[h] = np.concatenate([[0], np.cumsum(clen[h])[:-1]])
        half_len[h] = -(-int(clen[h].sum()) // 128) * 128  # pad tail to 128

    # block/window overlap map (compile-time, shared across cores):
    # mm list per window = blocks intersecting [coff, coff+clen)
    mm_cols = []   # global column order: for w: lo block uses, hi block uses
    win_mms = [[] for _ in range(nwin)]
    for w in range(nwin):
        for h in range(n_halves):
            a, b = int(coff[h, w]), int(coff[h, w] + clen[h, w])
            if b == a:
                continue
            for blk in range(a // 128, -(-b // 128)):
                win_mms[w].append((h, blk, len(mm_cols)))
                mm_cols.append((h, blk, w))
    nmm = len(mm_cols)

    # chunk split per half: warmup small, bulk CHUNK, cooldown small
    chunk_sizes = []
    for h in range(n_halves):
        rem, sizes = int(half_len[h]), []
        for wsz in WARM:
            L = min(wsz, rem)
            if L > 0:
                sizes.append(L)
                rem -= L
        while rem > 0:
            L = min(chunk, rem)
            sizes.append(L)
            rem -= L
        if sizes and sizes[-1] == chunk:
            sizes[-1:] = [chunk // 2, chunk // 4, chunk // 4]
        chunk_sizes.append(sizes)

    per_core_inputs = []
    for c in range(n_cores):
        s, d, cnt = per_core[c]
        offs = np.concatenate([[0], np.cumsum(cnt)])
        # slot streams per half: sources (idx) and per-slot local dst
        idx_stream = [np.zeros(int(half_len[h]), np.int64) for h in range(n_halves)]
        dst_stream = [np.full(int(half_len[h]), -1, np.int64) for h in range(n_halves)]
        for h in range(n_halves):
            for w in range(nwin):
                k = h * nwin + w
                a, b = int(offs[k]), int(offs[k + 1])
                o = int(coff[h, w])
                idx_stream[h][o:o + (b - a)] = s[a:b] - h * split
                dst_stream[h][o:o + (b - a)] = d[a:b] - c * npc - w * WIN

        # dstloc mask columns: one [128] column per (window, block) use;
        # slots outside the window's cell get -1
        dl = np.full((128, nmm), -1.0, np.float32)
        for col, (h, blk, w) in enumerate(mm_cols):
            s0, s1 = blk * 128, blk * 128 + 128
            a, b = int(coff[h, w]), int(coff[h, w] + clen[h, w])
            lo, hi = max(s0, a), min(s1, b)
            if hi > lo:
                seg = dst_stream[h][lo:hi].astype(np.float32)
                # mask slots whose dst is outside this window (pad slots
                # carry -1 already; straddle slots belong to w by range)
                dl[lo - s0:hi - s0, col] = seg
        core_in = {"dstloc": np.ascontiguousarray(dl)}

        # own-slab metadata for self-loops + output normalization
        nd = np.arange(npc, dtype=np.int64) + c * npc
        dv = dinv[nd]                                   # [npc]
        pad = nwin * WIN - npc
        dvp = np.concatenate([dv, np.zeros(pad, np.float32)])
        core_in["dinv_own"] = np.ascontiguousarray(
            dvp.reshape(nwin, WIN).T)                   # [128, nwin]
        core_in["rdinv"] = (1.0 / dvp.reshape(1, -1)[:, :npc].clip(1e-30)
                            ).astype(np.float32)        # [1, npc]

        # gather indices: wrapped [16, L/16] per chunk, replicated 8x
        for h in range(n_halves):
            cols, off = [], 0
            for L in chunk_sizes[h]:
                a = idx_stream[h][off:off + L].reshape(-1, 16).T
                cols.append(a)
                off += L
            wrapped = np.concatenate(cols, axis=1).astype(np.int16)
            core_in["idx_h%d" % h] = np.ascontiguousarray(
                np.tile(wrapped, (8, 1)))
        per_core_inputs.append(core_in)

    meta = dict(n_nodes=n_nodes, n_cores=n_cores, npc=npc, nwin=nwin,
                n_halves=n_halves, split=split, half_len=half_len,
                nmm=nmm, win_mms=win_mms, chunk=chunk,
                chunk_sizes=chunk_sizes, dinv=dinv)
    return meta, per_core_inputs


# ------------------------------------------------------------- bass program

def build_program(meta):
    import concourse.bacc as bacc
    import concourse.mybir as mybir
    import concourse.tile as tile
    from concourse import library_config

    f32 = mybir.dt.float32
    bf16 = mybir.dt.bfloat16
    i16 = mybir.dt.int16
    AF = mybir.ActivationFunctionType
    OP = mybir.AluOpType

    n_nodes = meta["n_nodes"]
    npc, nwin = meta["npc"], meta["nwin"]
    n_halves, split = meta["n_halves"], meta["split"]
    half_len = meta["half_len"]
    nmm = meta["nmm"]
    win_mms = meta["win_mms"]
    chunk = meta["chunk"]
    chunk_sizes = meta["chunk_sizes"]
    warm_tot = sum(WARM)

    nc = bacc.Bacc("TRN2", num_swdge_queues=N_QUEUES)

    x_d = nc.declare_dram_parameter("x", [n_nodes, D], bf16, isOutput=False)
    xo_d = nc.declare_dram_parameter("x_own", [npc, D], bf16, isOutput=False)
    dl_d = nc.declare_dram_parameter("dstloc", [128, nmm], bf16, isOutput=False)
    dvo_d = nc.declare_dram_parameter("dinv_own", [128, nwin], f32,
                                      isOutput=False)
    rd_d = nc.declare_dram_parameter("rdinv", [1, npc], bf16, isOutput=False)
    idx_d = [nc.declare_dram_parameter("idx_h%d" % h,
                                       [128, int(half_len[h]) // 16],
                                       i16, isOutput=False)
             for h in range(n_halves)]
    w1_d = nc.declare_dram_parameter("W1", [D, D], bf16, isOutput=False)
    w2_d = nc.declare_dram_parameter("W2", [D, D], bf16, isOutput=False)
    b1_d = nc.declare_dram_parameter("b1", [1, D], bf16, isOutput=False)
    b2_d = nc.declare_dram_parameter("b2", [1, D], bf16, isOutput=False)
    iota_d = nc.declare_dram_parameter("iota", [128, GF * 128], bf16,
                                       isOutput=False)
    id_d = nc.declare_dram_parameter("ident", [128, 128], bf16, isOutput=False)
    out_d = nc.declare_dram_parameter("out", [npc, D], f32, isOutput=True)

    WG = 4  # windows per phase-2 batch (one 512-wide psum bank)

    with tile.TileContext(nc) as tc:
        with (
            tc.tile_pool(name="const", bufs=1) as constp,
            tc.tile_pool(name="xg", bufs=12) as xgp,
            tc.tile_pool(name="eq", bufs=4) as eqp,
            tc.tile_pool(name="ps1", bufs=3, space="PSUM") as ps1,
            tc.tile_pool(name="ps2", bufs=2, space="PSUM") as ps2,
            tc.tile_pool(name="fin", bufs=3) as finp,
        ):
            # Q7 library holding DMAGatherAnt; must precede all gathers
            nc.gpsimd.load_library(library_config.mlp)

            # --- constants / metadata (weights and biases arrive already
            # scaled by 0.5; x rows arrive scaled by dinv[src])
            iota8 = constp.tile([128, GF, 128], bf16)
            nc.sync.dma_start(
                iota8[:], iota_d[:].rearrange("p (c n) -> p c n", n=128))
            ident = constp.tile([128, 128], bf16)
            nc.sync.dma_start(ident[:], id_d[:])
            wts = {}
            for nm, src_d in (("w1", w1_d), ("w2", w2_d)):
                t = constp.tile([128, 128], bf16, tag=nm)
                nc.sync.dma_start(t[:], src_d[:])
                wts[nm] = t
            bias = {}
            for nm, src_d in (("b1", b1_d), ("b2", b2_d)):
                t = constp.tile([1, 128], bf16, tag=nm)
                nc.sync.dma_start(t[:], src_d[:])
                bias[nm] = t
            rdinv = constp.tile([1, npc], bf16)
            nc.sync.dma_start(rdinv[:], rd_d[:])
            dvo = constp.tile([128, nwin], f32)
            nc.sync.dma_start(dvo[:], dvo_d[:])

            dl = constp.tile([128, nmm], bf16)
            nc.sync.dma_start(dl[:], dl_d[:])

            # own slab, node-major per window: [128 node, nwin, 128 feat]
            xown = constp.tile([128, nwin, 128], bf16)
            nc.sync.dma_start(
                xown[:, :npc // 128, :],
                xo_d[: (npc // 128) * 128, :].rearrange(
                    "(w p) f -> p w f", p=128))
            if npc % 128:
                nc.sync.dma_start(
                    xown[: npc % 128, npc // 128, :],
                    xo_d[(npc // 128) * 128:, :])

            g_all = constp.tile([128, npc], bf16)

            # idx streams: split the preload so the warmup chunks' indices
            # land quickly and the first gather starts early
            idx_all = []
            for h in range(n_halves):
                cols = int(half_len[h]) // 16
                wcols = min(warm_tot // 16, cols)
                t = constp.tile([128, cols], i16, tag="idx%d" % h)
                nc.sync.dma_start(t[:, :wcols], idx_d[h][:, :wcols])
                if wcols < cols:
                    nc.sync.dma_start(t[:, wcols:], idx_d[h][:, wcols:])
                idx_all.append(t)

            # one-hot columns: GF fused per DVE is_equal; consumed strictly
            # in column order so a single active group suffices
            eq_cache = [None, None]  # [group id, tile]

            def get_eq(col):
                g = col // GF
                if eq_cache[0] != g:
                    g0 = g * GF
                    gl = min(GF, nmm - g0)
                    eq = eqp.tile([128, GF, 128], bf16, tag="eq")
                    nc.vector.tensor_tensor(
                        out=eq[:, :gl, :], in0=iota8[:, :gl, :],
                        in1=dl[:, g0:g0 + gl, None].to_broadcast([128, gl, 128]),
                        op=OP.is_equal)
                    eq_cache[0] = g
                    eq_cache[1] = eq
                return eq_cache[1]

            # per-half stream state: lazy chunk issuing in window order
            class Stream:
                pass

            streams = []
            for h in range(n_halves):
                s = Stream()
                s.h = h
                s.base = x_d[0:split, :] if h == 0 else x_d[split:n_nodes, :]
                s.chunk_bounds = []
                off = 0
                for L in chunk_sizes[h]:
                    s.chunk_bounds.append((off, L))
                    off += L
                s.blk2chunk = np.repeat(
                    np.arange(len(chunk_sizes[h])),
                    [L // 128 for L in chunk_sizes[h]])
                s.tiles = {}
                streams.append(s)

            ci_global = 0

            def ensure_chunk(s, ci):
                nonlocal ci_global
                if ci in s.tiles:
                    return s.tiles[ci]
                off, L = s.chunk_bounds[ci]
                xg = xgp.tile([128, chunk // 128, 128], bf16, tag="xg")
                nc.gpsimd.dma_gather(
                    out_ap=xg[:, : L // 128, :],
                    in_ap=s.base,
                    idxs_ap=idx_all[s.h][:, off // 16:(off + L) // 16],
                    num_idxs=L,
                    num_idxs_reg=L,
                    elem_size=D,
                    single_packet=False,
                    queue_num=ci_global % N_QUEUES,
                )
                ci_global += 1
                s.tiles.clear()
                s.tiles[ci] = xg
                return xg

            # --- output stage: psum = g^T@(W/2) + rdinv*(b/2), then
            # relu with per-partition dst scale dinv[n]; layers averaged
            def emit_phase2(wlo, whi):
                nwg = whi - wlo + 1
                wls = [min(WIN, npc - w * WIN) for w in range(wlo, whi + 1)]
                pps = {}
                for nm_w, nm_b in (("w1", "b1"), ("w2", "b2")):
                    pp = ps2.tile([128, WG * 128], f32, tag="pp")
                    for j, w in enumerate(range(wlo, whi + 1)):
                        wl = wls[j]
                        sl = pp[:wl, j * 128:(j + 1) * 128]
                        nc.tensor.matmul(sl, g_all[:, w * WIN:w * WIN + wl],
                                         wts[nm_w][:], start=True, stop=False)
                        nc.tensor.matmul(sl,
                                         rdinv[:, w * WIN:w * WIN + wl],
                                         bias[nm_b][:], start=False, stop=True)
                    o = finp.tile([128, WG, 128], f32, tag="o" + nm_w)
                    for j, w in enumerate(range(wlo, whi + 1)):
                        nc.scalar.activation(
                            o[:wls[j], j, :],
                            pp[:wls[j], j * 128:(j + 1) * 128], AF.Relu,
                            scale=dvo[:wls[j], w:w + 1])
                    pps[nm_w] = o
                ot = finp.tile([128, WG, 128], f32, tag="ot")
                rows = min(wls)
                otf = ot[:].rearrange("p c n -> p (c n)")
                o1f = pps["w1"][:].rearrange("p c n -> p (c n)")
                o2f = pps["w2"][:].rearrange("p c n -> p (c n)")
                if rows == 128:
                    nc.vector.tensor_tensor(otf[:, :nwg * 128],
                                            o1f[:, :nwg * 128],
                                            o2f[:, :nwg * 128], op=OP.add)
                else:
                    for j in range(nwg):
                        cs = slice(j * 128, j * 128 + 128)
                        nc.vector.tensor_tensor(otf[:wls[j], cs],
                                                o1f[:wls[j], cs],
                                                o2f[:wls[j], cs], op=OP.add)
                for j, w in enumerate(range(wlo, whi + 1)):
                    nc.sync.dma_start(out_d[w * WIN:w * WIN + wls[j], :],
                                      ot[:wls[j], j, :])

            for w in range(nwin):
                wlen = min(WIN, npc - w * WIN)
                pw = ps1.tile([128, 128], f32, tag="pw")
                mms = win_mms[w]
                n_tot = len(mms) + 1
                # self-loop first: x'_own rows -> columns via identity
                nc.tensor.matmul(pw[:, :wlen], xown[:wlen, w, :],
                                 ident[:wlen, :wlen],
                                 start=True, stop=(n_tot == 1))
                for k, (h, blk, col) in enumerate(mms):
                    s = streams[h]
                    ci = int(s.blk2chunk[blk])
                    xg = ensure_chunk(s, ci)
                    bl = blk - s.chunk_bounds[ci][0] // 128
                    eq = get_eq(col)
                    nc.tensor.matmul(
                        pw[:, :wlen],
                        xg[:, bl, :],
                        eq[:, col % GF, :wlen],
                        start=False,
                        stop=(k == n_tot - 2),
                    )
                nc.scalar.activation(g_all[:, w * WIN:w * WIN + wlen],
                                     pw[:, :wlen], AF.Copy)
                if w % WG == WG - 1 or w == nwin - 1:
                    emit_phase2(w - (w % WG), w)

    nc.compile()
    return nc


def make_core_inputs(meta, per_core_inputs, x, W1, b1, W2, b2):
    """Full in_maps for run_bass_kernel_spmd (adds shared tensors).

    x rows are pre-scaled by dinv[src] so gathered rows carry the source
    normalization; weights/biases fold in the 0.5 layer average.
    """
    import ml_dtypes
    bf = ml_dtypes.bfloat16
    dinv = meta["dinv"]
    npc = meta["npc"]
    xs = (np.asarray(x, np.float32) * dinv[:, None]).astype(bf)
    xs = np.ascontiguousarray(xs)
    shared = {
        "x": xs,
        "W1": np.ascontiguousarray((0.5 * np.asarray(W1, np.float32)).astype(bf)),
        "W2": np.ascontiguousarray((0.5 * np.asarray(W2, np.float32)).astype(bf)),
        "b1": (0.5 * np.asarray(b1, np.float32)).astype(bf).reshape(1, D),
        "b2": (0.5 * np.asarray(b2, np.float32)).astype(bf).reshape(1, D),
        "iota": np.ascontiguousarray(np.broadcast_to(
            np.tile(np.arange(128, dtype=np.float32), GF),
            (128, GF * 128)).astype(bf)),
        "ident": np.ascontiguousarray(np.eye(128, dtype=np.float32).astype(bf)),
    }
    maps = []
    for c, ci in enumerate(per_core_inputs):
        m = dict(shared)
        m["x_own"] = np.ascontiguousarray(xs[c * npc:(c + 1) * npc, :])
        m["dstloc"] = np.ascontiguousarray(ci["dstloc"].astype(bf))
        m["dinv_own"] = np.ascontiguousarray(ci["dinv_own"].astype(np.float32))
        m["rdinv"] = np.ascontiguousarray(ci["rdinv"].astype(bf))
        for k, v in ci.items():
            if k.startswith("idx_"):
                m[k] = v
        maps.append(m)
    return maps


# ------------------------------------------------------------------- kernel

def kernel(x, edge_index, W1, b1, W2, b2, _trace=False):
    from concourse.bass_utils import run_bass_kernel_spmd

    x = np.asarray(x)
    n_nodes = x.shape[0]
    meta, pci = host_prep(edge_index, n_nodes, N_CORES)
    nc = build_program(meta)
    in_maps = make_core_inputs(meta, pci, x, W1, b1, W2, b2)
    res = run_bass_kernel_spmd(nc, in_maps, list(range(N_CORES)),
                               trace=_trace)
    out = np.concatenate([res.results[c]["out"] for c in range(N_CORES)],
                         axis=0)
    if _trace:
        return out, res
    return out


# revision 23
# speedup vs baseline: 1.2534x; 1.0340x over previous
"""Two-layer GCN (MultiOrderGraphLayer) Bass kernel for 8 Trainium2 cores.

Math: out = 0.5*(relu(A_hat@x@W1+b1) + relu(A_hat@x@W2+b2)) with
A_hat = D^-1/2 (A+I) D^-1/2.  Both layers share A_hat, so g = A_hat @ x is
computed once and the two small 128x128 matmuls are applied afterwards.

Normalization factorization: norm_e = dinv[src]*dinv[dst].  The host
pre-scales x rows by dinv (x' = dinv[i]*x[i], bf16), so gathered rows
already carry the src factor; the dst factor dinv[n] is applied in the
output stage as a per-partition activation scale (partition = node there).
Self-loops reduce to g_raw[:, n] += x'[n, :], added per window with one
identity matmul instead of gather slots.

Device algorithm (per core, feature-major g_T = [128 feat, nodes]):
  - nodes sharded 8 ways by row; edges partitioned by destination core and
    grouped per 128-node output window into variable-length cells packed
    back-to-back (cell length = max edge count across cores, so the SPMD
    program shape is shared; per-core shortfall is masked via dstloc=-1).
  - sources split into lo (<32768) / hi streams for int16 dma_gather.
  - the stream is consumed in fixed 128-slot blocks; for each (window,
    overlapping block) pair the host emits a dstloc column; slots outside
    the window (straddle or pad) carry -1 and the one-hot masks them out.
  - per block-use: build S[e, n] = (dstloc_e == n) in one DVE is_equal
    (bf16, GF columns fused); accumulate psum += xg^T @ S.
  - output: psum = g_T^T@(0.5W) + rdinv*(0.5b); out = relu(psum * dinv[n])
    (per-partition scale), layers averaged, written node-major.
"""

import math
import numpy as np

N_NODES = 50000
D = 128
N_CORES = 8
SPLIT = 32768  # int16 gather index limit
WIN = 128      # output-window size in nodes (one-hot width / psum free dim)
CHUNK = 2048   # slots per dma_gather instruction (multiple of 128); kept
               # small so two in-flight gathers per SWDGE queue fit the
               # ~16KB/partition descriptor ring without blocking emission
N_QUEUES = 4   # SWDGE queues (ucode max); rotation parallelizes drain
GF = 8         # one-hot columns fused per DVE is_equal op
WARM = (1024, 1024, 2048, 2048)  # warmup chunks; keep in sync with idx split


# ---------------------------------------------------------------- host prep

def host_prep(edge_index, n_nodes, n_cores, split=SPLIT, chunk=CHUNK):
    """Edge partitioning by destination core, per-window cells (variable
    length, shared shape across cores), lo/hi source split, block/window
    overlap map, dstloc mask columns.

    All heavy math stays on device; host work is indexing plus the x
    prescale (done in make_core_inputs).
    """
    src = np.asarray(edge_index[0], dtype=np.int64)
    dst = np.asarray(edge_index[1], dtype=np.int64)
    deg = np.bincount(dst, minlength=n_nodes).astype(np.int64) + 1
    dinv = (1.0 / np.sqrt(deg.astype(np.float64))).astype(np.float32)

    npc = n_nodes // n_cores
    assert npc * n_cores == n_nodes
    nwin = math.ceil(npc / WIN)
    n_halves = 2 if n_nodes > split else 1

    per_core = []
    counts = np.zeros((n_cores, n_halves, nwin), np.int64)
    for c in range(n_cores):
        n0 = c * npc
        m = (dst >= n0) & (dst < n0 + npc)
        s, d = src[m], dst[m]
        w = (d - n0) // WIN
        half = (s >= split).astype(np.int64) if n_halves == 2 else np.zeros_like(s)
        key = half * nwin + w
        order = np.argsort(key, kind="stable")
        s, d, key = s[order], d[order], key[order]
        cnt = np.bincount(key, minlength=n_halves * nwin)
        counts[c] = cnt.reshape(n_halves, nwin)
        per_core.append((s, d, cnt))

    # shared cell lengths (max across cores) and packed offsets per half
    clen = counts.max(axis=0)                       # [n_halves, nwin]
    coff = np.zeros_like(clen)
    half_len = np.zeros(n_halves, np.int64)
    for h in range(n_halves):
        coff[h] = np.concatenate([[0], np.cumsum(clen[h])[:-1]])
        half_len[h] = -(-int(clen[h].sum()) // 128) * 128  # pad tail to 128

    # block/window overlap map (compile-time, shared across cores):
    # mm list per window = blocks intersecting [coff, coff+clen)
    mm_cols = []   # global column order: for w: lo block uses, hi block uses
    win_mms = [[] for _ in range(nwin)]
    for w in range(nwin):
        for h in range(n_halves):
            a, b = int(coff[h, w]), int(coff[h, w] + clen[h, w])
            if b == a:
                continue
            for blk in range(a // 128, -(-b // 128)):
                win_mms[w].append((h, blk, len(mm_cols)))
                mm_cols.append((h, blk, w))
    nmm = len(mm_cols)

    # chunk split per half: warmup small, bulk CHUNK, cooldown small
    chunk_sizes = []
    for h in range(n_halves):
        rem, sizes = int(half_len[h]), []
        for wsz in WARM:
            L = min(wsz, rem)
            if L > 0:
                sizes.append(L)
                rem -= L
        while rem > 0:
            L = min(chunk, rem)
            sizes.append(L)
            rem -= L
        if sizes and sizes[-1] == chunk:
            sizes[-1:] = [chunk // 2, chunk // 4, chunk // 4]
        chunk_sizes.append(sizes)

    per_core_inputs = []
    for c in range(n_cores):
        s, d, cnt = per_core[c]
        offs = np.concatenate([[0], np.cumsum(cnt)])
        # slot streams per half: sources (idx) and per-slot local dst
        idx_stream = [np.zeros(int(half_len[h]), np.int64) for h in range(n_halves)]
        dst_stream = [np.full(int(half_len[h]), -1, np.int64) for h in range(n_halves)]
        for h in range(n_halves):
            for w in range(nwin):
                k = h * nwin + w
                a, b = int(offs[k]), int(offs[k + 1])
                o = int(coff[h, w])
                idx_stream[h][o:o + (b - a)] = s[a:b] - h * split
                dst_stream[h][o:o + (b - a)] = d[a:b] - c * npc - w * WIN

        # dstloc mask columns: one [128] column per (window, block) use;
        # slots outside the window's cell get -1
        dl = np.full((128, nmm), -1.0, np.float32)
        for col, (h, blk, w) in enumerate(mm_cols):
            s0, s1 = blk * 128, blk * 128 + 128
            a, b = int(coff[h, w]), int(coff[h, w] + clen[h, w])
            lo, hi = max(s0, a), min(s1, b)
            if hi > lo:
                seg = dst_stream[h][lo:hi].astype(np.float32)
                # mask slots whose dst is outside this window (pad slots
                # carry -1 already; straddle slots belong to w by range)
                dl[lo - s0:hi - s0, col] = seg
        core_in = {"dstloc": np.ascontiguousarray(dl)}

        # own-slab metadata for self-loops + output normalization
        nd = np.arange(npc, dtype=np.int64) + c * npc
        dv = dinv[nd]                                   # [npc]
        pad = nwin * WIN - npc
        dvp = np.concatenate([dv, np.zeros(pad, np.float32)])
        core_in["dinv_own"] = np.ascontiguousarray(
            dvp.reshape(nwin, WIN).T)                   # [128, nwin]
        core_in["rdinv"] = (1.0 / dvp.reshape(1, -1)[:, :npc].clip(1e-30)
                            ).astype(np.float32)        # [1, npc]

        # gather indices: wrapped [16, L/16] per chunk, replicated 8x
        for h in range(n_halves):
            cols, off = [], 0
            for L in chunk_sizes[h]:
                a = idx_stream[h][off:off + L].reshape(-1, 16).T
                cols.append(a)
                off += L
            wrapped = np.concatenate(cols, axis=1).astype(np.int16)
            core_in["idx_h%d" % h] = np.ascontiguousarray(
                np.tile(wrapped, (8, 1)))
        per_core_inputs.append(core_in)

    meta = dict(n_nodes=n_nodes, n_cores=n_cores, npc=npc, nwin=nwin,
                n_halves=n_halves, split=split, half_len=half_len,
                nmm=nmm, win_mms=win_mms, chunk=chunk,
                chunk_sizes=chunk_sizes, dinv=dinv)
    return meta, per_core_inputs


# ------------------------------------------------------------- bass program

def build_program(meta):
    import concourse.bacc as bacc
    import concourse.mybir as mybir
    import concourse.tile as tile
    from concourse import library_config

    f32 = mybir.dt.float32
    bf16 = mybir.dt.bfloat16
    i16 = mybir.dt.int16
    AF = mybir.ActivationFunctionType
    OP = mybir.AluOpType

    n_nodes = meta["n_nodes"]
    npc, nwin = meta["npc"], meta["nwin"]
    n_halves, split = meta["n_halves"], meta["split"]
    half_len = meta["half_len"]
    nmm = meta["nmm"]
    win_mms = meta["win_mms"]
    chunk = meta["chunk"]
    chunk_sizes = meta["chunk_sizes"]
    warm_tot = sum(WARM)

    nc = bacc.Bacc("TRN2", num_swdge_queues=N_QUEUES)

    x_d = nc.declare_dram_parameter("x", [n_nodes, D], bf16, isOutput=False)
    xo_d = nc.declare_dram_parameter("x_own", [npc, D], bf16, isOutput=False)
    dl_d = nc.declare_dram_parameter("dstloc", [128, nmm], bf16, isOutput=False)
    dvo_d = nc.declare_dram_parameter("dinv_own", [128, nwin], f32,
                                      isOutput=False)
    rd_d = nc.declare_dram_parameter("rdinv", [1, npc], bf16, isOutput=False)
    idx_d = [nc.declare_dram_parameter("idx_h%d" % h,
                                       [128, int(half_len[h]) // 16],
                                       i16, isOutput=False)
             for h in range(n_halves)]
    w1_d = nc.declare_dram_parameter("W1", [D, D], bf16, isOutput=False)
    w2_d = nc.declare_dram_parameter("W2", [D, D], bf16, isOutput=False)
    b1_d = nc.declare_dram_parameter("b1", [1, D], bf16, isOutput=False)
    b2_d = nc.declare_dram_parameter("b2", [1, D], bf16, isOutput=False)
    iota_d = nc.declare_dram_parameter("iota", [128, GF * 128], bf16,
                                       isOutput=False)
    id_d = nc.declare_dram_parameter("ident", [128, 128], bf16, isOutput=False)
    out_d = nc.declare_dram_parameter("out", [npc, D], f32, isOutput=True)

    WG = 4  # windows per phase-2 batch (one 512-wide psum bank)

    with tile.TileContext(nc) as tc:
        with (
            tc.tile_pool(name="const", bufs=1) as constp,
            tc.tile_pool(name="xg", bufs=12) as xgp,
            tc.tile_pool(name="eq", bufs=4) as eqp,
            tc.tile_pool(name="ps1", bufs=3, space="PSUM") as ps1,
            tc.tile_pool(name="ps2", bufs=2, space="PSUM") as ps2,
            tc.tile_pool(name="fin", bufs=3) as finp,
        ):
            # Q7 library holding DMAGatherAnt; must precede all gathers
            nc.gpsimd.load_library(library_config.mlp)

            # --- constants / metadata (weights and biases arrive already
            # scaled by 0.5; x rows arrive scaled by dinv[src])
            iota8 = constp.tile([128, GF, 128], bf16)
            nc.sync.dma_start(
                iota8[:], iota_d[:].rearrange("p (c n) -> p c n", n=128))
            ident = constp.tile([128, 128], bf16)
            nc.sync.dma_start(ident[:], id_d[:])
            wts = {}
            for nm, src_d in (("w1", w1_d), ("w2", w2_d)):
                t = constp.tile([128, 128], bf16, tag=nm)
                nc.sync.dma_start(t[:], src_d[:])
                wts[nm] = t
            bias = {}
            for nm, src_d in (("b1", b1_d), ("b2", b2_d)):
                t = constp.tile([1, 128], bf16, tag=nm)
                nc.sync.dma_start(t[:], src_d[:])
                bias[nm] = t
            rdinv = constp.tile([1, npc], bf16)
            nc.sync.dma_start(rdinv[:], rd_d[:])
            dvo = constp.tile([128, nwin], f32)
            nc.sync.dma_start(dvo[:], dvo_d[:])

            dl = constp.tile([128, nmm], bf16)
            nc.sync.dma_start(dl[:], dl_d[:])

            # warm idx slices in their OWN tiles right after dl: the first
            # gathers depend only on these small DMAs, not on the bulk idx
            # load or the 1.6MB x_own load below
            idx_warm = []
            warm_cols = sum(WARM) // 16
            for h in range(n_halves):
                cols = int(half_len[h]) // 16
                wcols = min(warm_cols, cols)
                tw = constp.tile([128, wcols], i16, tag="idxw%d" % h)
                nc.sync.dma_start(tw[:], idx_d[h][:, :wcols])
                idx_warm.append((tw, wcols))

            # own slab, node-major per window: [128 node, nwin, 128 feat]
            xown = constp.tile([128, nwin, 128], bf16)
            nc.sync.dma_start(
                xown[:, :npc // 128, :],
                xo_d[: (npc // 128) * 128, :].rearrange(
                    "(w p) f -> p w f", p=128))
            if npc % 128:
                nc.sync.dma_start(
                    xown[: npc % 128, npc // 128, :],
                    xo_d[(npc // 128) * 128:, :])

            g_all = constp.tile([128, npc], bf16)

            # bulk idx tail, separate tiles (issued after x_own)
            idx_rest = []
            for h in range(n_halves):
                cols = int(half_len[h]) // 16
                wcols = idx_warm[h][1]
                if wcols < cols:
                    tr = constp.tile([128, cols - wcols], i16,
                                     tag="idxr%d" % h)
                    nc.sync.dma_start(tr[:], idx_d[h][:, wcols:])
                    idx_rest.append((tr, wcols))
                else:
                    idx_rest.append(None)

            # one-hot columns: GF fused per DVE is_equal; consumed strictly
            # in column order so a single active group suffices
            eq_cache = [None, None]  # [group id, tile]

            def get_eq(col):
                g = col // GF
                if eq_cache[0] != g:
                    g0 = g * GF
                    gl = min(GF, nmm - g0)
                    eq = eqp.tile([128, GF, 128], bf16, tag="eq")
                    nc.vector.tensor_tensor(
                        out=eq[:, :gl, :], in0=iota8[:, :gl, :],
                        in1=dl[:, g0:g0 + gl, None].to_broadcast([128, gl, 128]),
                        op=OP.is_equal)
                    eq_cache[0] = g
                    eq_cache[1] = eq
                return eq_cache[1]

            # per-half stream state: lazy chunk issuing in window order
            class Stream:
                pass

            streams = []
            for h in range(n_halves):
                s = Stream()
                s.h = h
                s.base = x_d[0:split, :] if h == 0 else x_d[split:n_nodes, :]
                s.chunk_bounds = []
                off = 0
                for L in chunk_sizes[h]:
                    s.chunk_bounds.append((off, L))
                    off += L
                s.blk2chunk = np.repeat(
                    np.arange(len(chunk_sizes[h])),
                    [L // 128 for L in chunk_sizes[h]])
                s.tiles = {}
                streams.append(s)

            ci_global = 0

            def ensure_chunk(s, ci):
                nonlocal ci_global
                if ci in s.tiles:
                    return s.tiles[ci]
                off, L = s.chunk_bounds[ci]
                tw, wcols = idx_warm[s.h]
                if off // 16 < wcols:
                    assert (off + L) // 16 <= wcols, "chunk straddles warm"
                    it = tw[:, off // 16:(off + L) // 16]
                else:
                    tr, base = idx_rest[s.h]
                    it = tr[:, off // 16 - base:(off + L) // 16 - base]
                xg = xgp.tile([128, chunk // 128, 128], bf16, tag="xg")
                nc.gpsimd.dma_gather(
                    out_ap=xg[:, : L // 128, :],
                    in_ap=s.base,
                    idxs_ap=it,
                    num_idxs=L,
                    num_idxs_reg=L,
                    elem_size=D,
                    single_packet=False,
                    queue_num=ci_global % N_QUEUES,
                )
                ci_global += 1
                s.tiles.clear()
                s.tiles[ci] = xg
                return xg

            # --- output stage: psum = g^T@(W/2) + rdinv*(b/2), then
            # relu with per-partition dst scale dinv[n]; layers averaged
            def emit_phase2(wlo, whi):
                nwg = whi - wlo + 1
                wls = [min(WIN, npc - w * WIN) for w in range(wlo, whi + 1)]
                pps = {}
                for nm_w, nm_b in (("w1", "b1"), ("w2", "b2")):
                    pp = ps2.tile([128, WG * 128], f32, tag="pp")
                    for j, w in enumerate(range(wlo, whi + 1)):
                        wl = wls[j]
                        sl = pp[:wl, j * 128:(j + 1) * 128]
                        nc.tensor.matmul(sl, g_all[:, w * WIN:w * WIN + wl],
                                         wts[nm_w][:], start=True, stop=False)
                        nc.tensor.matmul(sl,
                                         rdinv[:, w * WIN:w * WIN + wl],
                                         bias[nm_b][:], start=False, stop=True)
                    o = finp.tile([128, WG, 128], f32, tag="o" + nm_w)
                    for j, w in enumerate(range(wlo, whi + 1)):
                        nc.scalar.activation(
                            o[:wls[j], j, :],
                            pp[:wls[j], j * 128:(j + 1) * 128], AF.Relu,
                            scale=dvo[:wls[j], w:w + 1])
                    pps[nm_w] = o
                ot = finp.tile([128, WG, 128], f32, tag="ot")
                rows = min(wls)
                otf = ot[:].rearrange("p c n -> p (c n)")
                o1f = pps["w1"][:].rearrange("p c n -> p (c n)")
                o2f = pps["w2"][:].rearrange("p c n -> p (c n)")
                if rows == 128:
                    nc.vector.tensor_tensor(otf[:, :nwg * 128],
                                            o1f[:, :nwg * 128],
                                            o2f[:, :nwg * 128], op=OP.add)
                else:
                    for j in range(nwg):
                        cs = slice(j * 128, j * 128 + 128)
                        nc.vector.tensor_tensor(otf[:wls[j], cs],
                                                o1f[:wls[j], cs],
                                                o2f[:wls[j], cs], op=OP.add)
                for j, w in enumerate(range(wlo, whi + 1)):
                    nc.sync.dma_start(out_d[w * WIN:w * WIN + wls[j], :],
                                      ot[:wls[j], j, :])

            for w in range(nwin):
                wlen = min(WIN, npc - w * WIN)
                pw = ps1.tile([128, 128], f32, tag="pw")
                mms = win_mms[w]
                n_tot = len(mms) + 1
                # self-loop first: x'_own rows -> columns via identity
                nc.tensor.matmul(pw[:, :wlen], xown[:wlen, w, :],
                                 ident[:wlen, :wlen],
                                 start=True, stop=(n_tot == 1))
                for k, (h, blk, col) in enumerate(mms):
                    s = streams[h]
                    ci = int(s.blk2chunk[blk])
                    xg = ensure_chunk(s, ci)
                    bl = blk - s.chunk_bounds[ci][0] // 128
                    eq = get_eq(col)
                    nc.tensor.matmul(
                        pw[:, :wlen],
                        xg[:, bl, :],
                        eq[:, col % GF, :wlen],
                        start=False,
                        stop=(k == n_tot - 2),
                    )
                nc.scalar.activation(g_all[:, w * WIN:w * WIN + wlen],
                                     pw[:, :wlen], AF.Copy)
                if w % WG == WG - 1 or w == nwin - 1:
                    emit_phase2(w - (w % WG), w)

    nc.compile()
    return nc


def make_core_inputs(meta, per_core_inputs, x, W1, b1, W2, b2):
    """Full in_maps for run_bass_kernel_spmd (adds shared tensors).

    x rows are pre-scaled by dinv[src] so gathered rows carry the source
    normalization; weights/biases fold in the 0.5 layer average.
    """
    import ml_dtypes
    bf = ml_dtypes.bfloat16
    dinv = meta["dinv"]
    npc = meta["npc"]
    xs = (np.asarray(x, np.float32) * dinv[:, None]).astype(bf)
    xs = np.ascontiguousarray(xs)
    shared = {
        "x": xs,
        "W1": np.ascontiguousarray((0.5 * np.asarray(W1, np.float32)).astype(bf)),
        "W2": np.ascontiguousarray((0.5 * np.asarray(W2, np.float32)).astype(bf)),
        "b1": (0.5 * np.asarray(b1, np.float32)).astype(bf).reshape(1, D),
        "b2": (0.5 * np.asarray(b2, np.float32)).astype(bf).reshape(1, D),
        "iota": np.ascontiguousarray(np.broadcast_to(
            np.tile(np.arange(128, dtype=np.float32), GF),
            (128, GF * 128)).astype(bf)),
        "ident": np.ascontiguousarray(np.eye(128, dtype=np.float32).astype(bf)),
    }
    maps = []
    for c, ci in enumerate(per_core_inputs):
        m = dict(shared)
        m["x_own"] = np.ascontiguousarray(xs[c * npc:(c + 1) * npc, :])
        m["dstloc"] = np.ascontiguousarray(ci["dstloc"].astype(bf))
        m["dinv_own"] = np.ascontiguousarray(ci["dinv_own"].astype(np.float32))
        m["rdinv"] = np.ascontiguousarray(ci["rdinv"].astype(bf))
        for k, v in ci.items():
            if k.startswith("idx_"):
                m[k] = v
        maps.append(m)
    return maps


# ------------------------------------------------------------------- kernel

def kernel(x, edge_index, W1, b1, W2, b2, _trace=False):
    from concourse.bass_utils import run_bass_kernel_spmd

    x = np.asarray(x)
    n_nodes = x.shape[0]
    meta, pci = host_prep(edge_index, n_nodes, N_CORES)
    nc = build_program(meta)
    in_maps = make_core_inputs(meta, pci, x, W1, b1, W2, b2)
    res = run_bass_kernel_spmd(nc, in_maps, list(range(N_CORES)),
                               trace=_trace)
    out = np.concatenate([res.results[c]["out"] for c in range(N_CORES)],
                         axis=0)
    if _trace:
        return out, res
    return out
